# revision 26
# baseline (speedup 1.0000x reference)
"""AnchorTarget (RPN anchor-target assignment) on 8 Trainium2 NeuronCores.

Strategy
--------
The dominant work is the (N=147456) x (G=256) IoU matrix plus row/column
max/argmax reductions.  Anchors are sharded across the 8 cores (18432 each =
144 tiles of 128 partitions).  Within a core, anchors are regrouped so each
tile holds anchors of a single base-anchor type (9 types x 16 tiles = 144):
anchor areas then take one value per tile, so S = areaA + areaB is
partition-constant per tile and 1/S is host-precomputed (9 x 256 broadcast
rows) — no division runs on the device at all.  Ordering uses
u = inter * (1/S), a monotone transform of IoU (u = iou/(1+iou)), which
preserves argmax, exact ties, and thresholds (iou >= c  <=>  u >= c/(1+c)).

Per core, fully fused in SBUF (the IoU matrix never touches HBM), pipelined
across three engines:
  * Pool/GpSimd: interval min/max terms + the intersection product
  * Scalar/ACT:  the +1/relu clamps and the two threshold Sign-sum counts
  * Vector/DVE:  widths, u, per-anchor rowmax + first-occurrence argmax
                 encode, and the per-(lane, gt) running column max/argmax

The host does the O(N+G) epilogue: global per-gt argmax combine (exact-tie
break by smallest original anchor index, matching jnp.argmax), label
assembly, the fixed-key (42) threefry subsampling (bits replicated on CPU
jax), inside-image masking, and the bbox-transform targets.
"""

import os
import numpy as np

# ---- problem constants (hardcoded; must match the reference) ----
N_CORES = 8
RR = CC = 128
NUM_BASE = 9
N = RR * CC * NUM_BASE          # 147456 anchors
NPC = N // N_CORES              # 18432 anchors per core
P = 128                         # partitions
TILES = NPC // P                # 144 tiles per core (= 9 types x 16)
TPK = TILES // NUM_BASE         # 16 tiles per anchor type
G = 256                         # gt boxes
STRIDE = 16
RPN_BATCHSIZE = 256
RPN_FG_FRACTION = 0.5
BIG = 16384.0                   # argmax encoding base (BIG - g), exact in f32
C03 = float(np.float32(0.3 / 1.3))   # iou >= 0.3  <=>  u >= 0.3/1.3
C07 = float(np.float32(0.7 / 1.7))

F32 = np.float32

KERNEL_EXEC_NS = None           # filled when tracing is enabled
KERNEL_PROFILE = None

# x-stripe sharding: global anchor ag -> pos = ag//9 (k = ag%9),
# iy = pos//128, jx = pos%128; core = jx//16 (16-column stripe),
# tile = k*16 + iy//8 (same y-rows on every core -> SPMD-uniform gt
# windows), partition = (iy%8)*16 + jx%16.
_AG = np.arange(N)
_POS = _AG // NUM_BASE
_KK = _AG % NUM_BASE
_IY = _POS // CC
_JX = _POS % CC
_C_OF = _JX // 16
_T_OF = _KK * TPK + _IY // 8
_P_OF = (_IY % 8) * 16 + _JX % 16
SEL = np.empty((N_CORES, TILES, P), np.int64)
SEL[_C_OF, _T_OF, _P_OF] = _AG


def _gt_windows(gt, base):
    """Per-tile contiguous gt ranges in y-sorted order.  Exclusion is exact
    (outside gts have zero IoU with every anchor of the tile); inclusion is
    conservative."""
    cy = (gt[:, 1] + gt[:, 3]) * 0.5
    pi = np.argsort(cy, kind="stable")
    gs = gt[pi]
    wins = []
    for t in range(TILES):
        k, blk = t // TPK, t % TPK
        ay1_min = 8 * blk * 16 + base[k, 1]
        ay2_max = (8 * blk + 7) * 16 + base[k, 3]
        incl = (gs[:, 3] >= ay1_min - 1.5) & (gs[:, 1] <= ay2_max + 1.5)
        idx = np.nonzero(incl)[0]
        if len(idx) == 0:
            lo, hi = 0, 64
        else:
            lo, hi = int(idx.min()), int(idx.max()) + 1
        # the Pool engine's chunked elementwise loop mishandles runs shorter
        # than its 64-element unroll -> keep every window at least 64 wide
        if hi - lo < 64:
            hi = min(G, lo + 64)
            lo = max(0, hi - 64)
        wins.append((lo, hi))
    return pi, wins


# ---------------------------------------------------------------- anchors --
def _base_anchors(base_size=16, ratios=(0.5, 1.0, 2.0), scales=(8.0, 16.0, 32.0)):
    ratios = np.asarray(ratios, np.float64)
    scales = np.asarray(scales, np.float64)
    base = np.array([1.0, 1.0, base_size, base_size]) - 1.0

    def whctrs(a):
        w = a[2] - a[0] + 1.0
        h = a[3] - a[1] + 1.0
        return w, h, a[0] + 0.5 * (w - 1.0), a[1] + 0.5 * (h - 1.0)

    def mk(ws, hs, xc, yc):
        return np.stack([xc - 0.5 * (ws - 1.0), yc - 0.5 * (hs - 1.0),
                         xc + 0.5 * (ws - 1.0), yc + 0.5 * (hs - 1.0)], axis=1)

    w, h, xc, yc = whctrs(base)
    size = w * h
    ws = np.round(np.sqrt(size / ratios))
    hs = np.round(ws * ratios)
    ratio_anchors = mk(ws, hs, xc, yc)
    out = []
    for ra in ratio_anchors:
        w2, h2, xc2, yc2 = whctrs(ra)
        out.append(mk(w2 * scales, h2 * scales, xc2, yc2))
    return np.concatenate(out, axis=0).astype(np.float32)  # (9, 4)


def _all_anchors():
    base = _base_anchors()                                  # f32 (9,4)
    sx = (np.arange(RR, dtype=np.float32) * np.float32(STRIDE))
    sy = (np.arange(CC, dtype=np.float32) * np.float32(STRIDE))
    SX, SY = np.meshgrid(sx, sy)                            # 'xy' like reference
    shifts = np.stack([SX.ravel(), SY.ravel(), SX.ravel(), SY.ravel()], axis=1)
    return (base[None, :, :] + shifts[:, None, :]).reshape(-1, 4).astype(np.float32)


# ------------------------------------------------------------ bass kernel --
def _build_bass(windows=None, tiles=TILES):
    """Raw-Bass build (explicit semaphores; the platform's codegen allows only
    one fused sync-wait per compute instruction, so the Tile framework's
    automatic semaphore insertion cannot be used).

    Four-stage software pipeline, skewed two tiles deep so cross-engine
    semaphore latency is hidden:
      DVE front(t):  interval min/max terms (4 tensor_scalar)
      GP  first(t):  raw widths iwr/ihr (tensor_tensor subtract)
      ACT front(t):  iw/ih = relu(+1)
      GP  second(t): inter = iw*ih,  u = inter * (1/S)
      ACT back(t):   threshold Sign-sum counts on u
      DVE back(t):   rowmax + argmax encode, column max/argmax chain
    All cross-engine tiles are triple-buffered (index t % 3)."""
    from contextlib import ExitStack

    import concourse.bass as bass
    from concourse import mybir

    dt = mybir.dt.float32
    op = mybir.AluOpType
    act = mybir.ActivationFunctionType
    if windows is None:
        windows = [(0, G)] * tiles

    nc = bass.Bass(detect_race_conditions=False)
    anch_p = nc.declare_dram_parameter("anch", [P, tiles * 4], dt, isOutput=False)
    gtb_p = nc.declare_dram_parameter("gtb", [P, (5 + NUM_BASE) * G], dt,
                                      isOutput=False)
    sgn03_p = nc.declare_dram_parameter("sgn03", [P, tiles], dt, isOutput=True)
    sgn07_p = nc.declare_dram_parameter("sgn07", [P, tiles], dt, isOutput=True)
    rowarg_p = nc.declare_dram_parameter("rowarg", [P, tiles], dt, isOutput=True)
    accval_p = nc.declare_dram_parameter("accval", [P, G], dt, isOutput=True)
    acct_p = nc.declare_dram_parameter("acct", [P, G], dt, isOutput=True)

    with ExitStack() as ctx:
        def sb(name, shape):
            return ctx.enter_context(nc.sbuf_tensor(name, shape, dt))
        anch_sb = sb("anch_sb", [P, tiles * 4])
        gtb_sb = sb("gtb_sb", [P, (5 + NUM_BASE) * G])
        accval = sb("accval_sb", [P, G])
        acct = sb("acct_sb", [P, G])
        sgn03_sb = sb("sgn03_sb", [P, tiles])
        sgn07_sb = sb("sgn07_sb", [P, tiles])
        rowarg_sb = sb("rowarg_sb", [P, tiles])
        NB = 4  # cross-engine buffers
        tminx = [sb(f"tminx{i}", [P, G]) for i in range(NB)]
        tmaxx = [sb(f"tmaxx{i}", [P, G]) for i in range(NB)]
        tminy = [sb(f"tminy{i}", [P, G]) for i in range(NB)]
        tmaxy = [sb(f"tmaxy{i}", [P, G]) for i in range(NB)]
        iwr = [sb(f"iwr{i}", [P, G]) for i in range(NB)]
        ihr = [sb(f"ihr{i}", [P, G]) for i in range(NB)]
        iw = [sb(f"iw{i}", [P, G]) for i in range(NB)]
        ih = [sb(f"ih{i}", [P, G]) for i in range(NB)]
        inter = [sb(f"inter{i}", [P, G]) for i in range(NB)]
        u = [sb(f"u{i}", [P, G]) for i in range(NB)]
        mask = sb("mask", [P, G])
        cmp_ = sb("cmp", [P, G])
        scr = sb("scr", [P, G])
        sgscr = sb("sgscr", [P, G])
        rq = sb("rq", [P, 1])
        b03 = sb("b03", [P, 1])
        b07 = sb("b07", [P, 1])

        dma_sem = ctx.enter_context(nc.semaphore("dma_sem"))
        s_dveA = ctx.enter_context(nc.semaphore("s_dveA"))  # DVE front(t) done
        s_gpw = ctx.enter_context(nc.semaphore("s_gpw"))    # GP iwr/ihr(t) done
        s_act1 = ctx.enter_context(nc.semaphore("s_act1"))  # ACT relus(t) done
        s_gpi = ctx.enter_context(nc.semaphore("s_gpi"))    # GP inter/u(t) done
        s_dveU = ctx.enter_context(nc.semaphore("s_dveU"))  # DVE back(t) done
        s_act2 = ctx.enter_context(nc.semaphore("s_act2"))  # ACT signs(t) done
        block = ctx.enter_context(nc.Block())

        gx1 = gtb_sb[:, 0 * G:1 * G]
        gy1 = gtb_sb[:, 1 * G:2 * G]
        gx2 = gtb_sb[:, 2 * G:3 * G]
        gy2 = gtb_sb[:, 3 * G:4 * G]

        @block.sync
        def _(sync):
            sync.dma_start(out=anch_sb[:], in_=anch_p[:]).then_inc(dma_sem, 16)
            sync.dma_start(out=gtb_sb[:], in_=gtb_p[:]).then_inc(dma_sem, 16)
            sync.wait_ge(s_dveU, tiles)
            sync.wait_ge(s_act2, tiles)
            sync.dma_start(out=rowarg_p[:], in_=rowarg_sb[:]).then_inc(dma_sem, 16)
            sync.dma_start(out=sgn03_p[:], in_=sgn03_sb[:]).then_inc(dma_sem, 16)
            sync.dma_start(out=sgn07_p[:], in_=sgn07_sb[:]).then_inc(dma_sem, 16)
            sync.dma_start(out=accval_p[:], in_=accval[:]).then_inc(dma_sem, 16)
            sync.dma_start(out=acct_p[:], in_=acct[:]).then_inc(dma_sem, 16)

        @block.gpsimd
        def _(gp):
            gp.wait_ge(dma_sem, 32)
            for t in range(tiles + 1):
                if t < tiles:
                    b = t % NB
                    lo, hi = windows[t]
                    w = hi - lo
                    gp.wait_ge(s_dveA, t + 1)
                    if t >= NB:
                        gp.wait_ge(s_act1, t - NB + 1)  # ACT read iwr of t-NB
                    gp.tensor_tensor(out=iwr[b][:, :w], in0=tminx[b][:, :w],
                                     in1=tmaxx[b][:, :w], op=op.subtract)
                    gp.tensor_tensor(out=ihr[b][:, :w], in0=tminy[b][:, :w],
                                     in1=tmaxy[b][:, :w],
                                     op=op.subtract).then_inc(s_gpw, 1)
                if t >= 1:
                    tq = t - 1
                    b = tq % NB
                    lo, hi = windows[tq]
                    w = hi - lo
                    k = tq // TPK
                    rsw = gtb_sb[:, (5 + k) * G + lo:(5 + k) * G + hi]
                    gp.wait_ge(s_act1, tq + 1)
                    if tq >= NB:
                        gp.wait_ge(s_dveU, tq - NB + 1)  # DVE read u of tq-NB
                        gp.wait_ge(s_act2, tq - NB + 1)  # ACT signed u of tq-NB
                    gp.tensor_tensor(out=inter[b][:, :w], in0=iw[b][:, :w],
                                     in1=ih[b][:, :w], op=op.mult)
                    gp.tensor_tensor(out=u[b][:, :w], in0=inter[b][:, :w],
                                     in1=rsw, op=op.mult).then_inc(s_gpi, 1)

        @block.vector
        def _(vector):
            vector.wait_ge(dma_sem, 32)
            vector.memset(accval[:], -1.0)
            vector.memset(acct[:], 0.0)
            vector.memset(b03[:], -C03)
            vector.memset(b07[:], -C07)
            for t in range(tiles + 2):
                if t < tiles:
                    b = t % NB
                    lo, hi = windows[t]
                    w = hi - lo
                    ax1 = anch_sb[:, t * 4 + 0:t * 4 + 1]
                    ay1 = anch_sb[:, t * 4 + 1:t * 4 + 2]
                    ax2 = anch_sb[:, t * 4 + 2:t * 4 + 3]
                    ay2 = anch_sb[:, t * 4 + 3:t * 4 + 4]
                    gx1w = gtb_sb[:, 0 * G + lo:0 * G + hi]
                    gy1w = gtb_sb[:, 1 * G + lo:1 * G + hi]
                    gx2w = gtb_sb[:, 2 * G + lo:2 * G + hi]
                    gy2w = gtb_sb[:, 3 * G + lo:3 * G + hi]
                    if t >= NB and t % NB == 0:
                        # one guard covers the next NB fronts: front(t+NB-1)
                        # needs GP first(t-1) done <=> s_gpw >= t
                        vector.wait_ge(s_gpw, t)
                    vector.tensor_scalar(out=tminx[b][:, :w], in0=gx2w,
                                         scalar1=ax2, scalar2=None, op0=op.min)
                    vector.tensor_scalar(out=tmaxx[b][:, :w], in0=gx1w,
                                         scalar1=ax1, scalar2=None, op0=op.max)
                    vector.tensor_scalar(out=tminy[b][:, :w], in0=gy2w,
                                         scalar1=ay2, scalar2=None, op0=op.min)
                    vector.tensor_scalar(out=tmaxy[b][:, :w], in0=gy1w,
                                         scalar1=ay1, scalar2=None,
                                         op0=op.max).then_inc(s_dveA, 1)
                if t >= 2:
                    tp = t - 2
                    b = tp % NB
                    lo, hi = windows[tp]
                    w = hi - lo
                    bmiw = gtb_sb[:, 4 * G + lo:4 * G + hi]
                    vector.wait_ge(s_gpi, tp + 1)
                    # NOTE: an op reading a per-partition AP scalar must not
                    # immediately follow the instruction that produced it (the
                    # scalar is prefetched before the producer drains on this
                    # hardware) -> cmp/acct separate the rq-reduce from the
                    # is_equal that consumes rq.
                    vector.tensor_reduce(out=rq[:], in_=u[b][:, :w],
                                         axis=mybir.AxisListType.X, op=op.max)
                    vector.tensor_tensor(out=cmp_[:, :w], in0=u[b][:, :w],
                                         in1=accval[:, lo:hi], op=op.is_gt)
                    vector.scalar_tensor_tensor(out=acct[:, lo:hi],
                                                in0=cmp_[:, :w],
                                                scalar=float(tp),
                                                in1=acct[:, lo:hi],
                                                op0=op.mult, op1=op.max)
                    vector.tensor_tensor(out=mask[:, :w], in0=u[b][:, :w],
                                         in1=rq[:, 0:1].to_broadcast((P, w)),
                                         op=op.is_equal)
                    vector.scalar_tensor_tensor(
                        out=scr[:, :w], in0=mask[:, :w], scalar=1.0, in1=bmiw,
                        op0=op.mult, op1=op.mult,
                        accum_out=rowarg_sb[:, tp:tp + 1])
                    vector.tensor_tensor(out=accval[:, lo:hi],
                                           in0=u[b][:, :w],
                                           in1=accval[:, lo:hi],
                                           op=op.max).then_inc(s_dveU, 1)

        @block.scalar
        def _(sc):
            for t in range(tiles + 1):
                if t < tiles:
                    b = t % NB
                    w = windows[t][1] - windows[t][0]
                    sc.wait_ge(s_gpw, t + 1)
                    sc.activation(out=iw[b][:, :w], in_=iwr[b][:, :w],
                                  func=act.Relu, bias=1.0, scale=1.0)
                    sc.activation(out=ih[b][:, :w], in_=ihr[b][:, :w],
                                  func=act.Relu, bias=1.0,
                                  scale=1.0).then_inc(s_act1, 1)
                if t >= 1:
                    tr = t - 1
                    b = tr % NB
                    w = windows[tr][1] - windows[tr][0]
                    sc.wait_ge(s_gpi, tr + 1)
                    sc.activation(out=sgscr[:, :w], in_=u[b][:, :w],
                                  func=act.Sign, bias=b03[:, 0:1], scale=1.0,
                                  accum_out=sgn03_sb[:, tr:tr + 1])
                    sc.activation(out=sgscr[:, :w], in_=u[b][:, :w],
                                  func=act.Sign, bias=b07[:, 0:1],
                                  scale=1.0,
                                  accum_out=sgn07_sb[:, tr:tr + 1]).then_inc(
                                      s_act2, 1)
            # final inc only via s_act2 (sync waits on the counting sems)

    return nc


# --------------------------------------------------------------- epilogue --
def _subsample(labels, target_value, max_count, U):
    is_t = labels == np.float32(target_value)
    pri = np.where(is_t, U, np.float32(-1.0)).astype(np.float32)
    order = np.argsort(-pri, kind="stable")
    rank = np.empty(labels.shape[0], np.int64)
    rank[order] = np.arange(labels.shape[0])
    drop = is_t & (rank >= max_count)
    return np.where(drop, np.float32(-1.0), labels).astype(np.float32)


def _bbox_transform(ex, gt):
    one = np.float32(1.0)
    half = np.float32(0.5)
    ew = ex[:, 2] - ex[:, 0] + one
    eh = ex[:, 3] - ex[:, 1] + one
    ecx = ex[:, 0] + half * ew
    ecy = ex[:, 1] + half * eh
    gw = gt[:, 2] - gt[:, 0] + one
    gh = gt[:, 3] - gt[:, 1] + one
    gcx = gt[:, 0] + half * gw
    gcy = gt[:, 1] + half * gh
    dx = (gcx - ecx) / ew
    dy = (gcy - ecy) / eh
    dw = np.log(gw / ew)
    dh = np.log(gh / eh)
    return np.stack([dx, dy, dw, dh], axis=1).astype(np.float32)


# ----------------------------------------------------------------- kernel --
def kernel(scores, gt_boxes, metadata, _trace=False):
    global KERNEL_EXEC_NS, KERNEL_PROFILE
    from concourse.bass_utils import run_bass_kernel_spmd

    trace = _trace or os.environ.get("ANCHOR_KERNEL_TRACE") == "1"
    if trace:
        try:
            import antenv.axon_hooks  # noqa: F401  (shimmed by test.py)
        except ImportError:
            trace = False

    gt = np.asarray(gt_boxes, F32)[0]                       # (256, 4)
    meta = np.asarray(metadata, F32)[0]

    anchors = _all_anchors()                                # (N, 4) f32
    base = _base_anchors()
    one = np.float32(1.0)
    area_a = ((anchors[:, 2] - anchors[:, 0] + one) *
              (anchors[:, 3] - anchors[:, 1] + one)).astype(F32)
    area_b = ((gt[:, 2] - gt[:, 0] + one) *
              (gt[:, 3] - gt[:, 1] + one)).astype(F32)
    A9 = area_a[:NUM_BASE]                                  # one area per type

    pi, wins = _gt_windows(gt, base)
    wt = np.array([hi - lo for lo, hi in wins], np.float32)  # per-tile widths

    # 1/S rows (gt axis permuted by pi), correctly rounded f32 of 1/fl(A_k+B_g)
    S9 = (A9[:, None] + area_b[None, pi]).astype(F32)       # (9, G)
    RS9 = (1.0 / S9.astype(np.float64)).astype(F32)
    gperm = gt[pi]
    bmi_row = (np.float32(BIG) - pi.astype(F32))            # BIG - original id

    grow = np.concatenate([gperm[:, 0], gperm[:, 1], gperm[:, 2], gperm[:, 3],
                           bmi_row, RS9.ravel()])
    gtb_arr = np.ascontiguousarray(
        np.broadcast_to(grow, (P, (5 + NUM_BASE) * G))).astype(F32)

    in_maps = []
    for c in range(N_CORES):
        asl = anchors[SEL[c]]                               # (TILES, P, 4)
        anch_arr = np.ascontiguousarray(
            asl.transpose(1, 0, 2).reshape(P, TILES * 4)).astype(F32)
        in_maps.append({"anch": anch_arr, "gtb": gtb_arr})

    nc = _build_bass(windows=wins)
    res = run_bass_kernel_spmd(nc, in_maps, core_ids=list(range(N_CORES)),
                               trace=trace)
    if trace:
        KERNEL_EXEC_NS = res.exec_time_ns
        KERNEL_PROFILE = res.profile_json
    outs = res.results

    def flat(name):
        # [128, TILES] per core, device position (c,t,p) -> global anchor
        X = np.empty(N, F32)
        for c in range(N_CORES):
            X[SEL[c].reshape(-1)] = np.asarray(outs[c][name]).T.ravel()
        return X

    sgn03 = flat("sgn03")
    sgn07 = flat("sgn07")
    rowarg_enc = flat("rowarg")
    # per-anchor window width (by its tile)
    wt_anchor = np.empty(N, F32)
    for c in range(N_CORES):
        wt_anchor[SEL[c].reshape(-1)] = np.repeat(wt, P)

    argmax_inds = np.rint(np.float32(BIG) - rowarg_enc).astype(np.int64)
    # enc outside the single-hot range means a tied (e.g. all-zero) row ->
    # first-occurrence argmax is 0
    argmax_inds[(argmax_inds < 0) | (argmax_inds > G - 1)] = 0

    # global per-gt argmax: max value, tie -> smallest original anchor index
    V = np.stack([np.asarray(outs[c]["accval"]) for c in range(N_CORES)])
    T = np.rint(np.stack([np.asarray(outs[c]["acct"])
                          for c in range(N_CORES)])).astype(np.int64)
    AIDX = np.empty((N_CORES, P, G), np.int64)
    for c in range(N_CORES):
        AIDX[c] = SEL[c][T[c], np.arange(P)[:, None]]
    best = V.max(axis=(0, 1))
    cand = np.where(V == best[None, None, :], AIDX, N)
    gt_argmax_dev = cand.min(axis=(0, 1))                   # indexed by pi-order
    gt_argmax = np.empty(G, np.int64)
    gt_argmax[pi] = gt_argmax_dev                           # original gt order

    # ---- label assembly (order follows the reference exactly) ----
    labels = np.full(N, -1.0, F32)
    labels = np.where(sgn03 <= -wt_anchor + 0.5, np.float32(0.0), labels)
    labels[gt_argmax] = 1.0
    labels = np.where(sgn07 > -wt_anchor + 0.5, np.float32(1.0), labels)

    import jax
    cpu = jax.devices("cpu")[0]
    with jax.default_device(cpu):
        k = jax.random.key(42)
        kpos, kneg = jax.random.split(k)
        U1 = np.asarray(jax.random.uniform(kpos, (N,)), F32)
        U2 = np.asarray(jax.random.uniform(kneg, (N,)), F32)

    num_fg = int(RPN_FG_FRACTION * RPN_BATCHSIZE)
    labels = _subsample(labels, 1.0, num_fg, U1)
    num_bg = RPN_BATCHSIZE - int(np.sum(labels == np.float32(1.0)))
    labels = _subsample(labels, 0.0, num_bg, U2)

    targets = _bbox_transform(anchors, gt[argmax_inds])

    inside = ((anchors[:, 0] >= 0) & (anchors[:, 1] >= 0) &
              (anchors[:, 2] < meta[1]) & (anchors[:, 3] < meta[0]))
    labels = np.where(inside, labels, np.float32(-1.0)).astype(F32)

    return labels[None, :], targets[None, :, :]


# revision 27
# speedup vs baseline: 1.0908x; 1.0908x over previous
"""AnchorTarget (RPN anchor-target assignment) on 8 Trainium2 NeuronCores.

Strategy
--------
The dominant work is the (N=147456) x (G=256) IoU matrix plus row/column
max/argmax reductions.  Anchors are sharded across the 8 cores (18432 each =
144 tiles of 128 partitions).  Within a core, anchors are regrouped so each
tile holds anchors of a single base-anchor type (9 types x 16 tiles = 144):
anchor areas then take one value per tile, so S = areaA + areaB is
partition-constant per tile and 1/S is host-precomputed (9 x 256 broadcast
rows) — no division runs on the device at all.  Ordering uses
u = inter * (1/S), a monotone transform of IoU (u = iou/(1+iou)), which
preserves argmax, exact ties, and thresholds (iou >= c  <=>  u >= c/(1+c)).

Per core, fully fused in SBUF (the IoU matrix never touches HBM), pipelined
across three engines:
  * Pool/GpSimd: interval min/max terms + the intersection product
  * Scalar/ACT:  the +1/relu clamps and the two threshold Sign-sum counts
  * Vector/DVE:  widths, u, per-anchor rowmax + first-occurrence argmax
                 encode, and the per-(lane, gt) running column max/argmax

The host does the O(N+G) epilogue: global per-gt argmax combine (exact-tie
break by smallest original anchor index, matching jnp.argmax), label
assembly, the fixed-key (42) threefry subsampling (bits replicated on CPU
jax), inside-image masking, and the bbox-transform targets.
"""

import os
import numpy as np

# ---- problem constants (hardcoded; must match the reference) ----
N_CORES = 8
RR = CC = 128
NUM_BASE = 9
N = RR * CC * NUM_BASE          # 147456 anchors
NPC = N // N_CORES              # 18432 anchors per core
P = 128                         # partitions
TILES = NPC // P                # 144 tiles per core (= 9 types x 16)
TPK = TILES // NUM_BASE         # 16 tiles per anchor type
G = 256                         # gt boxes
STRIDE = 16
RPN_BATCHSIZE = 256
RPN_FG_FRACTION = 0.5
BIG = 16384.0                   # argmax encoding base (BIG - g), exact in f32
C03 = float(np.float32(0.3 / 1.3))   # iou >= 0.3  <=>  u >= 0.3/1.3
C07 = float(np.float32(0.7 / 1.7))

F32 = np.float32

KERNEL_EXEC_NS = None           # filled when tracing is enabled
KERNEL_PROFILE = None

# x-stripe sharding: global anchor ag -> pos = ag//9 (k = ag%9),
# iy = pos//128, jx = pos%128; core = jx//16 (16-column stripe),
# tile = k*16 + iy//8 (same y-rows on every core -> SPMD-uniform gt
# windows), partition = (iy%8)*16 + jx%16.
_AG = np.arange(N)
_POS = _AG // NUM_BASE
_KK = _AG % NUM_BASE
_IY = _POS // CC
_JX = _POS % CC
_C_OF = _JX // 16
_T_OF = _KK * TPK + _IY // 8
_P_OF = (_IY % 8) * 16 + _JX % 16
SEL = np.empty((N_CORES, TILES, P), np.int64)
SEL[_C_OF, _T_OF, _P_OF] = _AG


def _gt_windows(gt, base):
    """Per-tile contiguous gt ranges in y-sorted order.  Exclusion is exact
    (outside gts have zero IoU with every anchor of the tile); inclusion is
    conservative."""
    cy = (gt[:, 1] + gt[:, 3]) * 0.5
    pi = np.argsort(cy, kind="stable")
    gs = gt[pi]
    wins = []
    for t in range(TILES):
        k, blk = t // TPK, t % TPK
        ay1_min = 8 * blk * 16 + base[k, 1]
        ay2_max = (8 * blk + 7) * 16 + base[k, 3]
        incl = (gs[:, 3] >= ay1_min - 1.5) & (gs[:, 1] <= ay2_max + 1.5)
        idx = np.nonzero(incl)[0]
        if len(idx) == 0:
            lo, hi = 0, 64
        else:
            lo, hi = int(idx.min()), int(idx.max()) + 1
        # the Pool engine's chunked elementwise loop mishandles runs shorter
        # than its 64-element unroll -> keep every window at least 64 wide
        if hi - lo < 64:
            hi = min(G, lo + 64)
            lo = max(0, hi - 64)
        wins.append((lo, hi))
    return pi, wins


# ---------------------------------------------------------------- anchors --
def _base_anchors(base_size=16, ratios=(0.5, 1.0, 2.0), scales=(8.0, 16.0, 32.0)):
    ratios = np.asarray(ratios, np.float64)
    scales = np.asarray(scales, np.float64)
    base = np.array([1.0, 1.0, base_size, base_size]) - 1.0

    def whctrs(a):
        w = a[2] - a[0] + 1.0
        h = a[3] - a[1] + 1.0
        return w, h, a[0] + 0.5 * (w - 1.0), a[1] + 0.5 * (h - 1.0)

    def mk(ws, hs, xc, yc):
        return np.stack([xc - 0.5 * (ws - 1.0), yc - 0.5 * (hs - 1.0),
                         xc + 0.5 * (ws - 1.0), yc + 0.5 * (hs - 1.0)], axis=1)

    w, h, xc, yc = whctrs(base)
    size = w * h
    ws = np.round(np.sqrt(size / ratios))
    hs = np.round(ws * ratios)
    ratio_anchors = mk(ws, hs, xc, yc)
    out = []
    for ra in ratio_anchors:
        w2, h2, xc2, yc2 = whctrs(ra)
        out.append(mk(w2 * scales, h2 * scales, xc2, yc2))
    return np.concatenate(out, axis=0).astype(np.float32)  # (9, 4)


def _all_anchors():
    base = _base_anchors()                                  # f32 (9,4)
    sx = (np.arange(RR, dtype=np.float32) * np.float32(STRIDE))
    sy = (np.arange(CC, dtype=np.float32) * np.float32(STRIDE))
    SX, SY = np.meshgrid(sx, sy)                            # 'xy' like reference
    shifts = np.stack([SX.ravel(), SY.ravel(), SX.ravel(), SY.ravel()], axis=1)
    return (base[None, :, :] + shifts[:, None, :]).reshape(-1, 4).astype(np.float32)


# ------------------------------------------------------------ bass kernel --
def _build_bass(windows=None, tiles=TILES):
    """Raw-Bass build (explicit semaphores; the platform's codegen allows only
    one fused sync-wait per compute instruction, so the Tile framework's
    automatic semaphore insertion cannot be used).

    Four-stage software pipeline, skewed two tiles deep so cross-engine
    semaphore latency is hidden:
      DVE front(t):  interval min/max terms (4 tensor_scalar)
      GP  first(t):  raw widths iwr/ihr (tensor_tensor subtract)
      ACT front(t):  iw/ih = relu(+1)
      GP  second(t): inter = iw*ih,  u = inter * (1/S)
      ACT back(t):   threshold Sign-sum counts on u
      DVE back(t):   rowmax + argmax encode, column max/argmax chain
    All cross-engine tiles are triple-buffered (index t % 3)."""
    from contextlib import ExitStack

    import concourse.bass as bass
    from concourse import mybir

    dt = mybir.dt.float32
    op = mybir.AluOpType
    act = mybir.ActivationFunctionType
    if windows is None:
        windows = [(0, G)] * tiles

    nc = bass.Bass(detect_race_conditions=False)
    anch_p = nc.declare_dram_parameter("anch", [P, tiles * 4], dt, isOutput=False)
    gtb_p = nc.declare_dram_parameter("gtb", [P, (5 + NUM_BASE) * G], dt,
                                      isOutput=False)
    sgn03_p = nc.declare_dram_parameter("sgn03", [P, tiles], dt, isOutput=True)
    sgn07_p = nc.declare_dram_parameter("sgn07", [P, tiles], dt, isOutput=True)
    rowarg_p = nc.declare_dram_parameter("rowarg", [P, tiles], dt, isOutput=True)
    accval_p = nc.declare_dram_parameter("accval", [P, G], dt, isOutput=True)
    acct_p = nc.declare_dram_parameter("acct", [P, G], dt, isOutput=True)

    with ExitStack() as ctx:
        def sb(name, shape):
            return ctx.enter_context(nc.sbuf_tensor(name, shape, dt))
        anch_sb = sb("anch_sb", [P, tiles * 4])
        gtb_sb = sb("gtb_sb", [P, (5 + NUM_BASE) * G])
        accval = sb("accval_sb", [P, G])
        acct = sb("acct_sb", [P, G])
        sgn03_sb = sb("sgn03_sb", [P, tiles])
        sgn07_sb = sb("sgn07_sb", [P, tiles])
        rowarg_sb = sb("rowarg_sb", [P, tiles])
        NB = 4  # cross-engine buffers
        # anchor x-coords depend only on the base-anchor type (not the y-block)
        # under x-stripe sharding -> the x min/max tables are per-type consts
        tminxK = [sb(f"tminxK{k}", [P, G]) for k in range(NUM_BASE)]
        tmaxxK = [sb(f"tmaxxK{k}", [P, G]) for k in range(NUM_BASE)]
        tminy = [sb(f"tminy{i}", [P, G]) for i in range(NB)]
        tmaxy = [sb(f"tmaxy{i}", [P, G]) for i in range(NB)]
        iwr = [sb(f"iwr{i}", [P, G]) for i in range(NB)]
        ihr = [sb(f"ihr{i}", [P, G]) for i in range(NB)]
        iw = [sb(f"iw{i}", [P, G]) for i in range(NB)]
        ih = [sb(f"ih{i}", [P, G]) for i in range(NB)]
        inter = [sb(f"inter{i}", [P, G]) for i in range(NB)]
        u = [sb(f"u{i}", [P, G]) for i in range(NB)]
        mask = sb("mask", [P, G])
        cmp_ = sb("cmp", [P, G])
        scr = sb("scr", [P, G])
        sgscr = sb("sgscr", [P, G])
        rq = sb("rq", [P, 1])
        b03 = sb("b03", [P, 1])
        b07 = sb("b07", [P, 1])

        dma_sem = ctx.enter_context(nc.semaphore("dma_sem"))
        s_dveA = ctx.enter_context(nc.semaphore("s_dveA"))  # DVE front(t) done
        s_gpw = ctx.enter_context(nc.semaphore("s_gpw"))    # GP iwr/ihr(t) done
        s_act1 = ctx.enter_context(nc.semaphore("s_act1"))  # ACT relus(t) done
        s_gpi = ctx.enter_context(nc.semaphore("s_gpi"))    # GP inter/u(t) done
        s_dveU = ctx.enter_context(nc.semaphore("s_dveU"))  # DVE back(t) done
        s_act2 = ctx.enter_context(nc.semaphore("s_act2"))  # ACT signs(t) done
        block = ctx.enter_context(nc.Block())

        gx1 = gtb_sb[:, 0 * G:1 * G]
        gy1 = gtb_sb[:, 1 * G:2 * G]
        gx2 = gtb_sb[:, 2 * G:3 * G]
        gy2 = gtb_sb[:, 3 * G:4 * G]

        @block.sync
        def _(sync):
            sync.dma_start(out=anch_sb[:], in_=anch_p[:]).then_inc(dma_sem, 16)
            sync.dma_start(out=gtb_sb[:], in_=gtb_p[:]).then_inc(dma_sem, 16)
            sync.wait_ge(s_dveU, tiles)
            sync.wait_ge(s_act2, tiles)
            sync.dma_start(out=rowarg_p[:], in_=rowarg_sb[:]).then_inc(dma_sem, 16)
            sync.dma_start(out=sgn03_p[:], in_=sgn03_sb[:]).then_inc(dma_sem, 16)
            sync.dma_start(out=sgn07_p[:], in_=sgn07_sb[:]).then_inc(dma_sem, 16)
            sync.dma_start(out=accval_p[:], in_=accval[:]).then_inc(dma_sem, 16)
            sync.dma_start(out=acct_p[:], in_=acct[:]).then_inc(dma_sem, 16)

        @block.gpsimd
        def _(gp):
            gp.wait_ge(dma_sem, 32)
            for t in range(tiles + 1):
                if t < tiles:
                    b = t % NB
                    lo, hi = windows[t]
                    w = hi - lo
                    k = t // TPK
                    gp.wait_ge(s_dveA, t + 1)
                    if t >= NB:
                        gp.wait_ge(s_act1, t - NB + 1)  # ACT read iwr of t-NB
                    gp.tensor_tensor(out=iwr[b][:, :w],
                                     in0=tminxK[k][:, lo:hi],
                                     in1=tmaxxK[k][:, lo:hi], op=op.subtract)
                    gp.tensor_tensor(out=ihr[b][:, :w], in0=tminy[b][:, :w],
                                     in1=tmaxy[b][:, :w],
                                     op=op.subtract).then_inc(s_gpw, 1)
                if t >= 1:
                    tq = t - 1
                    b = tq % NB
                    lo, hi = windows[tq]
                    w = hi - lo
                    k = tq // TPK
                    rsw = gtb_sb[:, (5 + k) * G + lo:(5 + k) * G + hi]
                    gp.wait_ge(s_act1, tq + 1)
                    if tq >= NB:
                        gp.wait_ge(s_dveU, tq - NB + 1)  # DVE read u of tq-NB
                        gp.wait_ge(s_act2, tq - NB + 1)  # ACT signed u of tq-NB
                    gp.tensor_tensor(out=inter[b][:, :w], in0=iw[b][:, :w],
                                     in1=ih[b][:, :w], op=op.mult)
                    gp.tensor_tensor(out=u[b][:, :w], in0=inter[b][:, :w],
                                     in1=rsw, op=op.mult).then_inc(s_gpi, 1)

        @block.vector
        def _(vector):
            vector.wait_ge(dma_sem, 32)
            vector.memset(accval[:], -1.0)
            vector.memset(acct[:], 0.0)
            vector.memset(b03[:], -C03)
            vector.memset(b07[:], -C07)
            gx1 = gtb_sb[:, 0 * G:1 * G]
            gx2 = gtb_sb[:, 2 * G:3 * G]
            for k in range(NUM_BASE):
                t0 = k * TPK
                ax1 = anch_sb[:, t0 * 4 + 0:t0 * 4 + 1]
                ax2 = anch_sb[:, t0 * 4 + 2:t0 * 4 + 3]
                vector.tensor_scalar(out=tminxK[k][:], in0=gx2, scalar1=ax2,
                                     scalar2=None, op0=op.min)
                vector.tensor_scalar(out=tmaxxK[k][:], in0=gx1, scalar1=ax1,
                                     scalar2=None, op0=op.max)
            for t in range(tiles + 2):
                if t < tiles:
                    b = t % NB
                    lo, hi = windows[t]
                    w = hi - lo
                    ay1 = anch_sb[:, t * 4 + 1:t * 4 + 2]
                    ay2 = anch_sb[:, t * 4 + 3:t * 4 + 4]
                    gy1w = gtb_sb[:, 1 * G + lo:1 * G + hi]
                    gy2w = gtb_sb[:, 3 * G + lo:3 * G + hi]
                    if t >= NB and t % NB == 0:
                        # one guard covers the next NB fronts: front(t+NB-1)
                        # needs GP first(t-1) done <=> s_gpw >= t
                        vector.wait_ge(s_gpw, t)
                    vector.tensor_scalar(out=tminy[b][:, :w], in0=gy2w,
                                         scalar1=ay2, scalar2=None, op0=op.min)
                    vector.tensor_scalar(out=tmaxy[b][:, :w], in0=gy1w,
                                         scalar1=ay1, scalar2=None,
                                         op0=op.max).then_inc(s_dveA, 1)
                if t >= 2:
                    tp = t - 2
                    b = tp % NB
                    lo, hi = windows[tp]
                    w = hi - lo
                    bmiw = gtb_sb[:, 4 * G + lo:4 * G + hi]
                    vector.wait_ge(s_gpi, tp + 1)
                    # NOTE: an op reading a per-partition AP scalar must not
                    # immediately follow the instruction that produced it (the
                    # scalar is prefetched before the producer drains on this
                    # hardware) -> cmp/acct separate the rq-reduce from the
                    # is_equal that consumes rq.
                    vector.tensor_reduce(out=rq[:], in_=u[b][:, :w],
                                         axis=mybir.AxisListType.X, op=op.max)
                    vector.tensor_tensor(out=cmp_[:, :w], in0=u[b][:, :w],
                                         in1=accval[:, lo:hi], op=op.is_gt)
                    vector.scalar_tensor_tensor(out=acct[:, lo:hi],
                                                in0=cmp_[:, :w],
                                                scalar=float(tp),
                                                in1=acct[:, lo:hi],
                                                op0=op.mult, op1=op.max)
                    vector.tensor_tensor(out=mask[:, :w], in0=u[b][:, :w],
                                         in1=rq[:, 0:1].to_broadcast((P, w)),
                                         op=op.is_equal)
                    vector.scalar_tensor_tensor(
                        out=scr[:, :w], in0=mask[:, :w], scalar=1.0, in1=bmiw,
                        op0=op.mult, op1=op.mult,
                        accum_out=rowarg_sb[:, tp:tp + 1])
                    vector.tensor_tensor(out=accval[:, lo:hi],
                                           in0=u[b][:, :w],
                                           in1=accval[:, lo:hi],
                                           op=op.max).then_inc(s_dveU, 1)

        @block.scalar
        def _(sc):
            for t in range(tiles + 1):
                if t < tiles:
                    b = t % NB
                    w = windows[t][1] - windows[t][0]
                    sc.wait_ge(s_gpw, t + 1)
                    sc.activation(out=iw[b][:, :w], in_=iwr[b][:, :w],
                                  func=act.Relu, bias=1.0, scale=1.0)
                    sc.activation(out=ih[b][:, :w], in_=ihr[b][:, :w],
                                  func=act.Relu, bias=1.0,
                                  scale=1.0).then_inc(s_act1, 1)
                if t >= 1:
                    tr = t - 1
                    b = tr % NB
                    w = windows[tr][1] - windows[tr][0]
                    sc.wait_ge(s_gpi, tr + 1)
                    sc.activation(out=sgscr[:, :w], in_=u[b][:, :w],
                                  func=act.Sign, bias=b03[:, 0:1], scale=1.0,
                                  accum_out=sgn03_sb[:, tr:tr + 1])
                    sc.activation(out=sgscr[:, :w], in_=u[b][:, :w],
                                  func=act.Sign, bias=b07[:, 0:1],
                                  scale=1.0,
                                  accum_out=sgn07_sb[:, tr:tr + 1]).then_inc(
                                      s_act2, 1)
            # final inc only via s_act2 (sync waits on the counting sems)

    return nc


# --------------------------------------------------------------- epilogue --
def _subsample(labels, target_value, max_count, U):
    is_t = labels == np.float32(target_value)
    pri = np.where(is_t, U, np.float32(-1.0)).astype(np.float32)
    order = np.argsort(-pri, kind="stable")
    rank = np.empty(labels.shape[0], np.int64)
    rank[order] = np.arange(labels.shape[0])
    drop = is_t & (rank >= max_count)
    return np.where(drop, np.float32(-1.0), labels).astype(np.float32)


def _bbox_transform(ex, gt):
    one = np.float32(1.0)
    half = np.float32(0.5)
    ew = ex[:, 2] - ex[:, 0] + one
    eh = ex[:, 3] - ex[:, 1] + one
    ecx = ex[:, 0] + half * ew
    ecy = ex[:, 1] + half * eh
    gw = gt[:, 2] - gt[:, 0] + one
    gh = gt[:, 3] - gt[:, 1] + one
    gcx = gt[:, 0] + half * gw
    gcy = gt[:, 1] + half * gh
    dx = (gcx - ecx) / ew
    dy = (gcy - ecy) / eh
    dw = np.log(gw / ew)
    dh = np.log(gh / eh)
    return np.stack([dx, dy, dw, dh], axis=1).astype(np.float32)


# ----------------------------------------------------------------- kernel --
def kernel(scores, gt_boxes, metadata, _trace=False):
    global KERNEL_EXEC_NS, KERNEL_PROFILE
    from concourse.bass_utils import run_bass_kernel_spmd

    trace = _trace or os.environ.get("ANCHOR_KERNEL_TRACE") == "1"
    if trace:
        try:
            import antenv.axon_hooks  # noqa: F401  (shimmed by test.py)
        except ImportError:
            trace = False

    gt = np.asarray(gt_boxes, F32)[0]                       # (256, 4)
    meta = np.asarray(metadata, F32)[0]

    anchors = _all_anchors()                                # (N, 4) f32
    base = _base_anchors()
    one = np.float32(1.0)
    area_a = ((anchors[:, 2] - anchors[:, 0] + one) *
              (anchors[:, 3] - anchors[:, 1] + one)).astype(F32)
    area_b = ((gt[:, 2] - gt[:, 0] + one) *
              (gt[:, 3] - gt[:, 1] + one)).astype(F32)
    A9 = area_a[:NUM_BASE]                                  # one area per type

    pi, wins = _gt_windows(gt, base)
    wt = np.array([hi - lo for lo, hi in wins], np.float32)  # per-tile widths

    # 1/S rows (gt axis permuted by pi), correctly rounded f32 of 1/fl(A_k+B_g)
    S9 = (A9[:, None] + area_b[None, pi]).astype(F32)       # (9, G)
    RS9 = (1.0 / S9.astype(np.float64)).astype(F32)
    gperm = gt[pi]
    bmi_row = (np.float32(BIG) - pi.astype(F32))            # BIG - original id

    grow = np.concatenate([gperm[:, 0], gperm[:, 1], gperm[:, 2], gperm[:, 3],
                           bmi_row, RS9.ravel()])
    gtb_arr = np.ascontiguousarray(
        np.broadcast_to(grow, (P, (5 + NUM_BASE) * G))).astype(F32)

    in_maps = []
    for c in range(N_CORES):
        asl = anchors[SEL[c]]                               # (TILES, P, 4)
        anch_arr = np.ascontiguousarray(
            asl.transpose(1, 0, 2).reshape(P, TILES * 4)).astype(F32)
        in_maps.append({"anch": anch_arr, "gtb": gtb_arr})

    nc = _build_bass(windows=wins)
    res = run_bass_kernel_spmd(nc, in_maps, core_ids=list(range(N_CORES)),
                               trace=trace)
    if trace:
        KERNEL_EXEC_NS = res.exec_time_ns
        KERNEL_PROFILE = res.profile_json
    outs = res.results

    def flat(name):
        # [128, TILES] per core, device position (c,t,p) -> global anchor
        X = np.empty(N, F32)
        for c in range(N_CORES):
            X[SEL[c].reshape(-1)] = np.asarray(outs[c][name]).T.ravel()
        return X

    sgn03 = flat("sgn03")
    sgn07 = flat("sgn07")
    rowarg_enc = flat("rowarg")
    # per-anchor window width (by its tile)
    wt_anchor = np.empty(N, F32)
    for c in range(N_CORES):
        wt_anchor[SEL[c].reshape(-1)] = np.repeat(wt, P)

    argmax_inds = np.rint(np.float32(BIG) - rowarg_enc).astype(np.int64)
    # enc outside the single-hot range means a tied (e.g. all-zero) row ->
    # first-occurrence argmax is 0
    argmax_inds[(argmax_inds < 0) | (argmax_inds > G - 1)] = 0

    # global per-gt argmax: max value, tie -> smallest original anchor index
    V = np.stack([np.asarray(outs[c]["accval"]) for c in range(N_CORES)])
    T = np.rint(np.stack([np.asarray(outs[c]["acct"])
                          for c in range(N_CORES)])).astype(np.int64)
    AIDX = np.empty((N_CORES, P, G), np.int64)
    for c in range(N_CORES):
        AIDX[c] = SEL[c][T[c], np.arange(P)[:, None]]
    best = V.max(axis=(0, 1))
    cand = np.where(V == best[None, None, :], AIDX, N)
    gt_argmax_dev = cand.min(axis=(0, 1))                   # indexed by pi-order
    gt_argmax = np.empty(G, np.int64)
    gt_argmax[pi] = gt_argmax_dev                           # original gt order

    # ---- label assembly (order follows the reference exactly) ----
    labels = np.full(N, -1.0, F32)
    labels = np.where(sgn03 <= -wt_anchor + 0.5, np.float32(0.0), labels)
    labels[gt_argmax] = 1.0
    labels = np.where(sgn07 > -wt_anchor + 0.5, np.float32(1.0), labels)

    import jax
    cpu = jax.devices("cpu")[0]
    with jax.default_device(cpu):
        k = jax.random.key(42)
        kpos, kneg = jax.random.split(k)
        U1 = np.asarray(jax.random.uniform(kpos, (N,)), F32)
        U2 = np.asarray(jax.random.uniform(kneg, (N,)), F32)

    num_fg = int(RPN_FG_FRACTION * RPN_BATCHSIZE)
    labels = _subsample(labels, 1.0, num_fg, U1)
    num_bg = RPN_BATCHSIZE - int(np.sum(labels == np.float32(1.0)))
    labels = _subsample(labels, 0.0, num_bg, U2)

    targets = _bbox_transform(anchors, gt[argmax_inds])

    inside = ((anchors[:, 0] >= 0) & (anchors[:, 1] >= 0) &
              (anchors[:, 2] < meta[1]) & (anchors[:, 3] < meta[0]))
    labels = np.where(inside, labels, np.float32(-1.0)).astype(F32)

    return labels[None, :], targets[None, :, :]


# revision 29
# speedup vs baseline: 1.1765x; 1.0786x over previous
"""AnchorTarget (RPN anchor-target assignment) on 8 Trainium2 NeuronCores.

Strategy
--------
The dominant work is the (N=147456) x (G=256) IoU matrix plus row/column
max/argmax reductions.  Anchors are sharded across the 8 cores (18432 each =
144 tiles of 128 partitions).  Within a core, anchors are regrouped so each
tile holds anchors of a single base-anchor type (9 types x 16 tiles = 144):
anchor areas then take one value per tile, so S = areaA + areaB is
partition-constant per tile and 1/S is host-precomputed (9 x 256 broadcast
rows) — no division runs on the device at all.  Ordering uses
u = inter * (1/S), a monotone transform of IoU (u = iou/(1+iou)), which
preserves argmax, exact ties, and thresholds (iou >= c  <=>  u >= c/(1+c)).

Per core, fully fused in SBUF (the IoU matrix never touches HBM), pipelined
across three engines:
  * Pool/GpSimd: interval min/max terms + the intersection product
  * Scalar/ACT:  the +1/relu clamps and the two threshold Sign-sum counts
  * Vector/DVE:  widths, u, per-anchor rowmax + first-occurrence argmax
                 encode, and the per-(lane, gt) running column max/argmax

The host does the O(N+G) epilogue: global per-gt argmax combine (exact-tie
break by smallest original anchor index, matching jnp.argmax), label
assembly, the fixed-key (42) threefry subsampling (bits replicated on CPU
jax), inside-image masking, and the bbox-transform targets.
"""

import os
import numpy as np

# ---- problem constants (hardcoded; must match the reference) ----
N_CORES = 8
RR = CC = 128
NUM_BASE = 9
N = RR * CC * NUM_BASE          # 147456 anchors
NPC = N // N_CORES              # 18432 anchors per core
P = 128                         # partitions
TILES = NPC // P                # 144 tiles per core (= 9 types x 16)
TPK = TILES // NUM_BASE         # 16 tiles per anchor type
G = 256                         # gt boxes
STRIDE = 16
RPN_BATCHSIZE = 256
RPN_FG_FRACTION = 0.5
BIG = 16384.0                   # argmax encoding base (BIG - g), exact in f32
C03 = float(np.float32(0.3 / 1.3))   # iou >= 0.3  <=>  u >= 0.3/1.3
C07 = float(np.float32(0.7 / 1.7))

F32 = np.float32

KERNEL_EXEC_NS = None           # filled when tracing is enabled
KERNEL_PROFILE = None

# x-stripe sharding: global anchor ag -> pos = ag//9 (k = ag%9),
# iy = pos//128, jx = pos%128; core = jx//16 (16-column stripe),
# tile = k*16 + iy//8 (same y-rows on every core -> SPMD-uniform gt
# windows), partition = (iy%8)*16 + jx%16.
_AG = np.arange(N)
_POS = _AG // NUM_BASE
_KK = _AG % NUM_BASE
_IY = _POS // CC
_JX = _POS % CC
_C_OF = _JX // 16
_T_OF = _KK * TPK + _IY // 8
_P_OF = (_IY % 8) * 16 + _JX % 16
SEL = np.empty((N_CORES, TILES, P), np.int64)
SEL[_C_OF, _T_OF, _P_OF] = _AG


def _gt_windows(gt, base):
    """Per-tile contiguous gt ranges in y-sorted order.  Exclusion is exact
    (outside gts have zero IoU with every anchor of the tile); inclusion is
    conservative."""
    cy = (gt[:, 1] + gt[:, 3]) * 0.5
    pi = np.argsort(cy, kind="stable")
    gs = gt[pi]
    wins = []
    for t in range(TILES):
        k, blk = t // TPK, t % TPK
        ay1_min = 8 * blk * 16 + base[k, 1]
        ay2_max = (8 * blk + 7) * 16 + base[k, 3]
        incl = (gs[:, 3] >= ay1_min - 1.5) & (gs[:, 1] <= ay2_max + 1.5)
        idx = np.nonzero(incl)[0]
        if len(idx) == 0:
            lo, hi = 0, 64
        else:
            lo, hi = int(idx.min()), int(idx.max()) + 1
        # the Pool engine's chunked elementwise loop mishandles runs shorter
        # than its 64-element unroll -> keep every window at least 64 wide
        if hi - lo < 64:
            hi = min(G, lo + 64)
            lo = max(0, hi - 64)
        wins.append((lo, hi))
    return pi, wins


# ---------------------------------------------------------------- anchors --
def _base_anchors(base_size=16, ratios=(0.5, 1.0, 2.0), scales=(8.0, 16.0, 32.0)):
    ratios = np.asarray(ratios, np.float64)
    scales = np.asarray(scales, np.float64)
    base = np.array([1.0, 1.0, base_size, base_size]) - 1.0

    def whctrs(a):
        w = a[2] - a[0] + 1.0
        h = a[3] - a[1] + 1.0
        return w, h, a[0] + 0.5 * (w - 1.0), a[1] + 0.5 * (h - 1.0)

    def mk(ws, hs, xc, yc):
        return np.stack([xc - 0.5 * (ws - 1.0), yc - 0.5 * (hs - 1.0),
                         xc + 0.5 * (ws - 1.0), yc + 0.5 * (hs - 1.0)], axis=1)

    w, h, xc, yc = whctrs(base)
    size = w * h
    ws = np.round(np.sqrt(size / ratios))
    hs = np.round(ws * ratios)
    ratio_anchors = mk(ws, hs, xc, yc)
    out = []
    for ra in ratio_anchors:
        w2, h2, xc2, yc2 = whctrs(ra)
        out.append(mk(w2 * scales, h2 * scales, xc2, yc2))
    return np.concatenate(out, axis=0).astype(np.float32)  # (9, 4)


def _all_anchors():
    base = _base_anchors()                                  # f32 (9,4)
    sx = (np.arange(RR, dtype=np.float32) * np.float32(STRIDE))
    sy = (np.arange(CC, dtype=np.float32) * np.float32(STRIDE))
    SX, SY = np.meshgrid(sx, sy)                            # 'xy' like reference
    shifts = np.stack([SX.ravel(), SY.ravel(), SX.ravel(), SY.ravel()], axis=1)
    return (base[None, :, :] + shifts[:, None, :]).reshape(-1, 4).astype(np.float32)


# ------------------------------------------------------------ bass kernel --
def _build_bass(windows=None, tiles=TILES):
    """Raw-Bass build (explicit semaphores; the platform's codegen allows only
    one fused sync-wait per compute instruction, so the Tile framework's
    automatic semaphore insertion cannot be used).

    Four-stage software pipeline, skewed two tiles deep so cross-engine
    semaphore latency is hidden:
      DVE front(t):  interval min/max terms (4 tensor_scalar)
      GP  first(t):  raw widths iwr/ihr (tensor_tensor subtract)
      ACT front(t):  iw/ih = relu(+1)
      GP  second(t): inter = iw*ih,  u = inter * (1/S)
      ACT back(t):   threshold Sign-sum counts on u
      DVE back(t):   rowmax + argmax encode, column max/argmax chain
    All cross-engine tiles are triple-buffered (index t % 3)."""
    from contextlib import ExitStack

    import concourse.bass as bass
    from concourse import mybir

    dt = mybir.dt.float32
    op = mybir.AluOpType
    act = mybir.ActivationFunctionType
    if windows is None:
        windows = [(0, G)] * tiles

    nc = bass.Bass(detect_race_conditions=False)
    anch_p = nc.declare_dram_parameter("anch", [P, tiles * 4], dt, isOutput=False)
    gtb_p = nc.declare_dram_parameter("gtb", [P, (5 + NUM_BASE) * G], dt,
                                      isOutput=False)
    sgn03_p = nc.declare_dram_parameter("sgn03", [P, tiles], dt, isOutput=True)
    sgn07_p = nc.declare_dram_parameter("sgn07", [P, tiles], dt, isOutput=True)
    rowarg_p = nc.declare_dram_parameter("rowarg", [P, tiles], dt, isOutput=True)
    accval_p = nc.declare_dram_parameter("accval", [P, G], dt, isOutput=True)
    acct_p = nc.declare_dram_parameter("acct", [P, G], dt, isOutput=True)

    with ExitStack() as ctx:
        def sb(name, shape):
            return ctx.enter_context(nc.sbuf_tensor(name, shape, dt))
        anch_sb = sb("anch_sb", [P, tiles * 4])
        gtb_sb = sb("gtb_sb", [P, (5 + NUM_BASE) * G])
        accval = sb("accval_sb", [P, G])
        acct = sb("acct_sb", [P, G])
        sgn03_sb = sb("sgn03_sb", [P, tiles])
        sgn07_sb = sb("sgn07_sb", [P, tiles])
        rowarg_sb = sb("rowarg_sb", [P, tiles])
        NB = 4  # cross-engine buffers
        # anchor x-coords depend only on the base-anchor type (not the y-block)
        # under x-stripe sharding -> the x min/max tables are per-type consts
        tminxK = [sb(f"tminxK{k}", [P, G]) for k in range(NUM_BASE)]
        tmaxxK = [sb(f"tmaxxK{k}", [P, G]) for k in range(NUM_BASE)]
        tminy = [sb(f"tminy{i}", [P, G]) for i in range(NB)]
        tmaxy = [sb(f"tmaxy{i}", [P, G]) for i in range(NB)]
        iwr = [sb(f"iwr{i}", [P, G]) for i in range(NB)]
        ihr = [sb(f"ihr{i}", [P, G]) for i in range(NB)]
        iw = [sb(f"iw{i}", [P, G]) for i in range(NB)]
        ih = [sb(f"ih{i}", [P, G]) for i in range(NB)]
        inter = [sb(f"inter{i}", [P, G]) for i in range(NB)]
        u = [sb(f"u{i}", [P, G]) for i in range(NB)]
        masks = [sb(f"masks{i}", [P, G]) for i in range(NB)]
        nrq = [sb(f"nrq{i}", [P, 1]) for i in range(NB)]
        cmp_ = sb("cmp", [P, G])
        scr = sb("scr", [P, G])
        sgscr = sb("sgscr", [P, G])
        rq = sb("rq", [P, 1])
        b03 = sb("b03", [P, 1])
        b07 = sb("b07", [P, 1])

        dma_sem = ctx.enter_context(nc.semaphore("dma_sem"))
        s_dveA = ctx.enter_context(nc.semaphore("s_dveA"))  # DVE front(t) done
        s_gpw = ctx.enter_context(nc.semaphore("s_gpw"))    # GP iwr/ihr(t) done
        s_act1 = ctx.enter_context(nc.semaphore("s_act1"))  # ACT relus(t) done
        s_gpi = ctx.enter_context(nc.semaphore("s_gpi"))    # GP inter/u(t) done
        s_dveU = ctx.enter_context(nc.semaphore("s_dveU"))  # DVE back(t) done
        s_act2 = ctx.enter_context(nc.semaphore("s_act2"))  # ACT signs(t) done
        s_dveR = ctx.enter_context(nc.semaphore("s_dveR"))  # DVE nrq(t) ready
        s_act3 = ctx.enter_context(nc.semaphore("s_act3"))  # ACT masksign(t) done
        s_dveE = ctx.enter_context(nc.semaphore("s_dveE"))  # DVE enc(t) done
        block = ctx.enter_context(nc.Block())

        gx1 = gtb_sb[:, 0 * G:1 * G]
        gy1 = gtb_sb[:, 1 * G:2 * G]
        gx2 = gtb_sb[:, 2 * G:3 * G]
        gy2 = gtb_sb[:, 3 * G:4 * G]

        @block.sync
        def _(sync):
            sync.dma_start(out=anch_sb[:], in_=anch_p[:]).then_inc(dma_sem, 16)
            sync.dma_start(out=gtb_sb[:], in_=gtb_p[:]).then_inc(dma_sem, 16)
            sync.wait_ge(s_dveU, tiles)
            sync.wait_ge(s_act2, tiles)
            sync.wait_ge(s_dveE, tiles)
            sync.dma_start(out=rowarg_p[:], in_=rowarg_sb[:]).then_inc(dma_sem, 16)
            sync.dma_start(out=sgn03_p[:], in_=sgn03_sb[:]).then_inc(dma_sem, 16)
            sync.dma_start(out=sgn07_p[:], in_=sgn07_sb[:]).then_inc(dma_sem, 16)
            sync.dma_start(out=accval_p[:], in_=accval[:]).then_inc(dma_sem, 16)
            sync.dma_start(out=acct_p[:], in_=acct[:]).then_inc(dma_sem, 16)

        @block.gpsimd
        def _(gp):
            gp.wait_ge(dma_sem, 32)
            for t in range(tiles + 1):
                if t < tiles:
                    b = t % NB
                    lo, hi = windows[t]
                    w = hi - lo
                    k = t // TPK
                    gp.wait_ge(s_dveA, t + 1)
                    if t >= NB:
                        gp.wait_ge(s_act1, t - NB + 1)  # ACT read iwr of t-NB
                    gp.tensor_tensor(out=iwr[b][:, :w],
                                     in0=tminxK[k][:, lo:hi],
                                     in1=tmaxxK[k][:, lo:hi], op=op.subtract)
                    gp.tensor_tensor(out=ihr[b][:, :w], in0=tminy[b][:, :w],
                                     in1=tmaxy[b][:, :w],
                                     op=op.subtract).then_inc(s_gpw, 1)
                if t >= 1:
                    tq = t - 1
                    b = tq % NB
                    lo, hi = windows[tq]
                    w = hi - lo
                    k = tq // TPK
                    rsw = gtb_sb[:, (5 + k) * G + lo:(5 + k) * G + hi]
                    gp.wait_ge(s_act1, tq + 1)
                    if tq >= NB:
                        gp.wait_ge(s_dveU, tq - NB + 1)  # DVE read u of tq-NB
                        gp.wait_ge(s_act2, tq - NB + 1)  # ACT signed u of tq-NB
                    gp.tensor_tensor(out=inter[b][:, :w], in0=iw[b][:, :w],
                                     in1=ih[b][:, :w], op=op.mult)
                    gp.tensor_tensor(out=u[b][:, :w], in0=inter[b][:, :w],
                                     in1=rsw, op=op.mult).then_inc(s_gpi, 1)

        @block.vector
        def _(vector):
            vector.wait_ge(dma_sem, 32)
            vector.memset(accval[:], -1.0)
            vector.memset(acct[:], 0.0)
            vector.memset(b03[:], -C03)
            vector.memset(b07[:], -C07)
            gx1 = gtb_sb[:, 0 * G:1 * G]
            gx2 = gtb_sb[:, 2 * G:3 * G]
            for k in range(NUM_BASE):
                t0 = k * TPK
                ax1 = anch_sb[:, t0 * 4 + 0:t0 * 4 + 1]
                ax2 = anch_sb[:, t0 * 4 + 2:t0 * 4 + 3]
                vector.tensor_scalar(out=tminxK[k][:], in0=gx2, scalar1=ax2,
                                     scalar2=None, op0=op.min)
                vector.tensor_scalar(out=tmaxxK[k][:], in0=gx1, scalar1=ax1,
                                     scalar2=None, op0=op.max)
            for t in range(tiles + 3):
                if t < tiles:
                    b = t % NB
                    lo, hi = windows[t]
                    w = hi - lo
                    ay1 = anch_sb[:, t * 4 + 1:t * 4 + 2]
                    ay2 = anch_sb[:, t * 4 + 3:t * 4 + 4]
                    gy1w = gtb_sb[:, 1 * G + lo:1 * G + hi]
                    gy2w = gtb_sb[:, 3 * G + lo:3 * G + hi]
                    if t >= NB and t % NB == 0:
                        # one guard covers the next NB fronts: front(t+NB-1)
                        # needs GP first(t-1) done <=> s_gpw >= t
                        vector.wait_ge(s_gpw, t)
                    vector.tensor_scalar(out=tminy[b][:, :w], in0=gy2w,
                                         scalar1=ay2, scalar2=None, op0=op.min)
                    vector.tensor_scalar(out=tmaxy[b][:, :w], in0=gy1w,
                                         scalar1=ay1, scalar2=None,
                                         op0=op.max).then_inc(s_dveA, 1)
                if 2 <= t < tiles + 2:
                    tp = t - 2
                    b = tp % NB
                    lo, hi = windows[tp]
                    w = hi - lo
                    vector.wait_ge(s_gpi, tp + 1)
                    # NOTE: an op reading a per-partition AP scalar must not
                    # immediately follow the instruction that produced it (the
                    # scalar is prefetched before the producer drains on this
                    # hardware) -> cmp separates the rq-reduce from the
                    # negation that consumes rq.
                    vector.tensor_reduce(out=rq[:], in_=u[b][:, :w],
                                         axis=mybir.AxisListType.X, op=op.max)
                    vector.tensor_tensor(out=cmp_[:, :w], in0=u[b][:, :w],
                                         in1=accval[:, lo:hi], op=op.is_gt)
                    vector.tensor_scalar(out=nrq[b][:], in0=rq[:], scalar1=-1.0,
                                         scalar2=None,
                                         op0=op.mult).then_inc(s_dveR, 1)
                    vector.scalar_tensor_tensor(out=acct[:, lo:hi],
                                                in0=cmp_[:, :w],
                                                scalar=float(tp),
                                                in1=acct[:, lo:hi],
                                                op0=op.mult, op1=op.max)
                    vector.tensor_tensor(out=accval[:, lo:hi],
                                           in0=u[b][:, :w],
                                           in1=accval[:, lo:hi],
                                           op=op.max).then_inc(s_dveU, 1)
                if t >= 3:
                    te = t - 3
                    be = te % NB
                    lo, hi = windows[te]
                    w = hi - lo
                    bmiw = gtb_sb[:, 4 * G + lo:4 * G + hi]
                    vector.wait_ge(s_act3, te + 1)
                    # enc = sum(sign(u-rowmax) * bmi); host adds the window's
                    # bmi sum (sign is {-1,0}, zero exactly at the rowmax)
                    vector.scalar_tensor_tensor(
                        out=scr[:, :w], in0=masks[be][:, :w], scalar=1.0,
                        in1=bmiw, op0=op.mult, op1=op.mult,
                        accum_out=rowarg_sb[:, te:te + 1]).then_inc(s_dveE, 1)

        @block.scalar
        def _(sc):
            for t in range(tiles + 1):
                if t < tiles:
                    b = t % NB
                    w = windows[t][1] - windows[t][0]
                    sc.wait_ge(s_gpw, t + 1)
                    sc.activation(out=iw[b][:, :w], in_=iwr[b][:, :w],
                                  func=act.Relu, bias=1.0, scale=1.0)
                    sc.activation(out=ih[b][:, :w], in_=ihr[b][:, :w],
                                  func=act.Relu, bias=1.0,
                                  scale=1.0).then_inc(s_act1, 1)
                if t >= 1:
                    tr = t - 1
                    b = tr % NB
                    w = windows[tr][1] - windows[tr][0]
                    sc.wait_ge(s_gpi, tr + 1)
                    sc.wait_ge(s_dveR, tr + 1)
                    if tr >= NB:
                        sc.wait_ge(s_dveE, tr - NB + 1)  # enc read masks(tr-NB)
                    sc.activation(out=masks[b][:, :w], in_=u[b][:, :w],
                                  func=act.Sign, bias=nrq[b][:, 0:1],
                                  scale=1.0).then_inc(s_act3, 1)
                    sc.activation(out=sgscr[:, :w], in_=u[b][:, :w],
                                  func=act.Sign, bias=b03[:, 0:1], scale=1.0,
                                  accum_out=sgn03_sb[:, tr:tr + 1])
                    sc.activation(out=sgscr[:, :w], in_=u[b][:, :w],
                                  func=act.Sign, bias=b07[:, 0:1],
                                  scale=1.0,
                                  accum_out=sgn07_sb[:, tr:tr + 1]).then_inc(
                                      s_act2, 1)
            # final inc only via s_act2 (sync waits on the counting sems)

    return nc


# --------------------------------------------------------------- epilogue --
def _subsample(labels, target_value, max_count, U):
    is_t = labels == np.float32(target_value)
    pri = np.where(is_t, U, np.float32(-1.0)).astype(np.float32)
    order = np.argsort(-pri, kind="stable")
    rank = np.empty(labels.shape[0], np.int64)
    rank[order] = np.arange(labels.shape[0])
    drop = is_t & (rank >= max_count)
    return np.where(drop, np.float32(-1.0), labels).astype(np.float32)


def _bbox_transform(ex, gt):
    one = np.float32(1.0)
    half = np.float32(0.5)
    ew = ex[:, 2] - ex[:, 0] + one
    eh = ex[:, 3] - ex[:, 1] + one
    ecx = ex[:, 0] + half * ew
    ecy = ex[:, 1] + half * eh
    gw = gt[:, 2] - gt[:, 0] + one
    gh = gt[:, 3] - gt[:, 1] + one
    gcx = gt[:, 0] + half * gw
    gcy = gt[:, 1] + half * gh
    dx = (gcx - ecx) / ew
    dy = (gcy - ecy) / eh
    dw = np.log(gw / ew)
    dh = np.log(gh / eh)
    return np.stack([dx, dy, dw, dh], axis=1).astype(np.float32)


# ----------------------------------------------------------------- kernel --
def kernel(scores, gt_boxes, metadata, _trace=False):
    global KERNEL_EXEC_NS, KERNEL_PROFILE
    from concourse.bass_utils import run_bass_kernel_spmd

    trace = _trace or os.environ.get("ANCHOR_KERNEL_TRACE") == "1"
    if trace:
        try:
            import antenv.axon_hooks  # noqa: F401  (shimmed by test.py)
        except ImportError:
            trace = False

    gt = np.asarray(gt_boxes, F32)[0]                       # (256, 4)
    meta = np.asarray(metadata, F32)[0]

    anchors = _all_anchors()                                # (N, 4) f32
    base = _base_anchors()
    one = np.float32(1.0)
    area_a = ((anchors[:, 2] - anchors[:, 0] + one) *
              (anchors[:, 3] - anchors[:, 1] + one)).astype(F32)
    area_b = ((gt[:, 2] - gt[:, 0] + one) *
              (gt[:, 3] - gt[:, 1] + one)).astype(F32)
    A9 = area_a[:NUM_BASE]                                  # one area per type

    pi, wins = _gt_windows(gt, base)
    wt = np.array([hi - lo for lo, hi in wins], np.float32)  # per-tile widths

    # 1/S rows (gt axis permuted by pi), correctly rounded f32 of 1/fl(A_k+B_g)
    S9 = (A9[:, None] + area_b[None, pi]).astype(F32)       # (9, G)
    RS9 = (1.0 / S9.astype(np.float64)).astype(F32)
    gperm = gt[pi]
    bmi_row = (np.float32(BIG) - pi.astype(F32))            # BIG - original id

    grow = np.concatenate([gperm[:, 0], gperm[:, 1], gperm[:, 2], gperm[:, 3],
                           bmi_row, RS9.ravel()])
    gtb_arr = np.ascontiguousarray(
        np.broadcast_to(grow, (P, (5 + NUM_BASE) * G))).astype(F32)

    in_maps = []
    for c in range(N_CORES):
        asl = anchors[SEL[c]]                               # (TILES, P, 4)
        anch_arr = np.ascontiguousarray(
            asl.transpose(1, 0, 2).reshape(P, TILES * 4)).astype(F32)
        in_maps.append({"anch": anch_arr, "gtb": gtb_arr})

    nc = _build_bass(windows=wins)
    res = run_bass_kernel_spmd(nc, in_maps, core_ids=list(range(N_CORES)),
                               trace=trace)
    if trace:
        KERNEL_EXEC_NS = res.exec_time_ns
        KERNEL_PROFILE = res.profile_json
    outs = res.results

    def flat(name):
        # [128, TILES] per core, device position (c,t,p) -> global anchor
        X = np.empty(N, F32)
        for c in range(N_CORES):
            X[SEL[c].reshape(-1)] = np.asarray(outs[c][name]).T.ravel()
        return X

    sgn03 = flat("sgn03")
    sgn07 = flat("sgn07")
    # device encodes sum(sign(u - rowmax) * bmi) with sign in {-1, 0};
    # adding the window's bmi sum recovers the single-hot BIG - gt_id code
    sb_tile = np.array([bmi_row[lo:hi].astype(np.float64).sum()
                        for lo, hi in wins])
    sb_anchor = np.empty(N, np.float64)
    for c in range(N_CORES):
        sb_anchor[SEL[c].reshape(-1)] = np.repeat(sb_tile, P)
    rowarg_enc = flat("rowarg").astype(np.float64) + sb_anchor
    # per-anchor window width (by its tile)
    wt_anchor = np.empty(N, F32)
    for c in range(N_CORES):
        wt_anchor[SEL[c].reshape(-1)] = np.repeat(wt, P)

    argmax_inds = np.rint(np.float32(BIG) - rowarg_enc).astype(np.int64)
    # enc outside the single-hot range means a tied (e.g. all-zero) row ->
    # first-occurrence argmax is 0
    argmax_inds[(argmax_inds < 0) | (argmax_inds > G - 1)] = 0

    # global per-gt argmax: max value, tie -> smallest original anchor index
    V = np.stack([np.asarray(outs[c]["accval"]) for c in range(N_CORES)])
    T = np.rint(np.stack([np.asarray(outs[c]["acct"])
                          for c in range(N_CORES)])).astype(np.int64)
    AIDX = np.empty((N_CORES, P, G), np.int64)
    for c in range(N_CORES):
        AIDX[c] = SEL[c][T[c], np.arange(P)[:, None]]
    best = V.max(axis=(0, 1))
    cand = np.where(V == best[None, None, :], AIDX, N)
    gt_argmax_dev = cand.min(axis=(0, 1))                   # indexed by pi-order
    gt_argmax = np.empty(G, np.int64)
    gt_argmax[pi] = gt_argmax_dev                           # original gt order

    # ---- label assembly (order follows the reference exactly) ----
    labels = np.full(N, -1.0, F32)
    labels = np.where(sgn03 <= -wt_anchor + 0.5, np.float32(0.0), labels)
    labels[gt_argmax] = 1.0
    labels = np.where(sgn07 > -wt_anchor + 0.5, np.float32(1.0), labels)

    import jax
    cpu = jax.devices("cpu")[0]
    with jax.default_device(cpu):
        k = jax.random.key(42)
        kpos, kneg = jax.random.split(k)
        U1 = np.asarray(jax.random.uniform(kpos, (N,)), F32)
        U2 = np.asarray(jax.random.uniform(kneg, (N,)), F32)

    num_fg = int(RPN_FG_FRACTION * RPN_BATCHSIZE)
    labels = _subsample(labels, 1.0, num_fg, U1)
    num_bg = RPN_BATCHSIZE - int(np.sum(labels == np.float32(1.0)))
    labels = _subsample(labels, 0.0, num_bg, U2)

    targets = _bbox_transform(anchors, gt[argmax_inds])

    inside = ((anchors[:, 0] >= 0) & (anchors[:, 1] >= 0) &
              (anchors[:, 2] < meta[1]) & (anchors[:, 3] < meta[0]))
    labels = np.where(inside, labels, np.float32(-1.0)).astype(F32)

    return labels[None, :], targets[None, :, :]


# revision 30
# speedup vs baseline: 1.4145x; 1.2023x over previous
"""AnchorTarget (RPN anchor-target assignment) on 8 Trainium2 NeuronCores.

Strategy
--------
The dominant work is the (N=147456) x (G=256) IoU matrix plus row/column
max/argmax reductions.  Anchors are sharded across the 8 cores (18432 each =
144 tiles of 128 partitions).  Within a core, anchors are regrouped so each
tile holds anchors of a single base-anchor type (9 types x 16 tiles = 144):
anchor areas then take one value per tile, so S = areaA + areaB is
partition-constant per tile and 1/S is host-precomputed (9 x 256 broadcast
rows) — no division runs on the device at all.  Ordering uses
u = inter * (1/S), a monotone transform of IoU (u = iou/(1+iou)), which
preserves argmax, exact ties, and thresholds (iou >= c  <=>  u >= c/(1+c)).

Per core, fully fused in SBUF (the IoU matrix never touches HBM), pipelined
across three engines:
  * Pool/GpSimd: interval min/max terms + the intersection product
  * Scalar/ACT:  the +1/relu clamps and the two threshold Sign-sum counts
  * Vector/DVE:  widths, u, per-anchor rowmax + first-occurrence argmax
                 encode, and the per-(lane, gt) running column max/argmax

The host does the O(N+G) epilogue: global per-gt argmax combine (exact-tie
break by smallest original anchor index, matching jnp.argmax), label
assembly, the fixed-key (42) threefry subsampling (bits replicated on CPU
jax), inside-image masking, and the bbox-transform targets.
"""

import os
import numpy as np

# ---- problem constants (hardcoded; must match the reference) ----
N_CORES = 8
RR = CC = 128
NUM_BASE = 9
N = RR * CC * NUM_BASE          # 147456 anchors
NPC = N // N_CORES              # 18432 anchors per core
P = 128                         # partitions
TILES = NPC // P                # 144 tiles per core (= 9 types x 16)
TPK = TILES // NUM_BASE         # 16 tiles per anchor type
G = 256                         # gt boxes
STRIDE = 16
RPN_BATCHSIZE = 256
RPN_FG_FRACTION = 0.5
BIG = 16384.0                   # argmax encoding base (BIG - g), exact in f32
C03 = float(np.float32(0.3 / 1.3))   # iou >= 0.3  <=>  u >= 0.3/1.3
C07 = float(np.float32(0.7 / 1.7))

F32 = np.float32

KERNEL_EXEC_NS = None           # filled when tracing is enabled
KERNEL_PROFILE = None

# x-stripe sharding: global anchor ag -> pos = ag//9 (k = ag%9),
# iy = pos//128, jx = pos%128; core = jx//16 (16-column stripe),
# tile = k*16 + iy//8 (same y-rows on every core -> SPMD-uniform gt
# windows), partition = (iy%8)*16 + jx%16.
_AG = np.arange(N)
_POS = _AG // NUM_BASE
_KK = _AG % NUM_BASE
_IY = _POS // CC
_JX = _POS % CC
_C_OF = _JX // 16
_T_OF = _KK * TPK + _IY // 8
_P_OF = (_IY % 8) * 16 + _JX % 16
SEL = np.empty((N_CORES, TILES, P), np.int64)
SEL[_C_OF, _T_OF, _P_OF] = _AG


def _gt_windows(gt, base):
    """Per-tile contiguous gt ranges in y-sorted order.  Exclusion is exact
    (outside gts have zero IoU with every anchor of the tile); inclusion is
    conservative."""
    cy = (gt[:, 1] + gt[:, 3]) * 0.5
    pi = np.argsort(cy, kind="stable")
    gs = gt[pi]
    wins = []
    for t in range(TILES):
        k, blk = t // TPK, t % TPK
        ay1_min = 8 * blk * 16 + base[k, 1]
        ay2_max = (8 * blk + 7) * 16 + base[k, 3]
        incl = (gs[:, 3] >= ay1_min - 1.5) & (gs[:, 1] <= ay2_max + 1.5)
        idx = np.nonzero(incl)[0]
        if len(idx) == 0:
            lo, hi = 0, 64
        else:
            lo, hi = int(idx.min()), int(idx.max()) + 1
        # the Pool engine's chunked elementwise loop mishandles runs shorter
        # than its 64-element unroll -> keep every window at least 64 wide
        if hi - lo < 64:
            hi = min(G, lo + 64)
            lo = max(0, hi - 64)
        wins.append((lo, hi))
    return pi, wins


# ---------------------------------------------------------------- anchors --
def _base_anchors(base_size=16, ratios=(0.5, 1.0, 2.0), scales=(8.0, 16.0, 32.0)):
    ratios = np.asarray(ratios, np.float64)
    scales = np.asarray(scales, np.float64)
    base = np.array([1.0, 1.0, base_size, base_size]) - 1.0

    def whctrs(a):
        w = a[2] - a[0] + 1.0
        h = a[3] - a[1] + 1.0
        return w, h, a[0] + 0.5 * (w - 1.0), a[1] + 0.5 * (h - 1.0)

    def mk(ws, hs, xc, yc):
        return np.stack([xc - 0.5 * (ws - 1.0), yc - 0.5 * (hs - 1.0),
                         xc + 0.5 * (ws - 1.0), yc + 0.5 * (hs - 1.0)], axis=1)

    w, h, xc, yc = whctrs(base)
    size = w * h
    ws = np.round(np.sqrt(size / ratios))
    hs = np.round(ws * ratios)
    ratio_anchors = mk(ws, hs, xc, yc)
    out = []
    for ra in ratio_anchors:
        w2, h2, xc2, yc2 = whctrs(ra)
        out.append(mk(w2 * scales, h2 * scales, xc2, yc2))
    return np.concatenate(out, axis=0).astype(np.float32)  # (9, 4)


def _all_anchors():
    base = _base_anchors()                                  # f32 (9,4)
    sx = (np.arange(RR, dtype=np.float32) * np.float32(STRIDE))
    sy = (np.arange(CC, dtype=np.float32) * np.float32(STRIDE))
    SX, SY = np.meshgrid(sx, sy)                            # 'xy' like reference
    shifts = np.stack([SX.ravel(), SY.ravel(), SX.ravel(), SY.ravel()], axis=1)
    return (base[None, :, :] + shifts[:, None, :]).reshape(-1, 4).astype(np.float32)


# ------------------------------------------------------------ bass kernel --
def _build_bass(windows=None, tiles=TILES):
    """Raw-Bass build (explicit semaphores; the platform's codegen allows only
    one fused sync-wait per compute instruction, so the Tile framework's
    automatic semaphore insertion cannot be used).

    Four-stage software pipeline, skewed two tiles deep so cross-engine
    semaphore latency is hidden:
      DVE front(t):  interval min/max terms (4 tensor_scalar)
      GP  first(t):  raw widths iwr/ihr (tensor_tensor subtract)
      ACT front(t):  iw/ih = relu(+1)
      GP  second(t): inter = iw*ih,  u = inter * (1/S)
      ACT back(t):   threshold Sign-sum counts on u
      DVE back(t):   rowmax + argmax encode, column max/argmax chain
    All cross-engine tiles are triple-buffered (index t % 3)."""
    from contextlib import ExitStack

    import concourse.bass as bass
    from concourse import mybir

    dt = mybir.dt.float32
    op = mybir.AluOpType
    act = mybir.ActivationFunctionType
    if windows is None:
        windows = [(0, G)] * tiles

    nc = bass.Bass(detect_race_conditions=False)
    anch_p = nc.declare_dram_parameter("anch", [P, tiles * 4], dt, isOutput=False)
    gtb_p = nc.declare_dram_parameter("gtb", [P, (5 + NUM_BASE) * G], dt,
                                      isOutput=False)
    sgn03_p = nc.declare_dram_parameter("sgn03", [P, tiles], dt, isOutput=True)
    sgn07_p = nc.declare_dram_parameter("sgn07", [P, tiles], dt, isOutput=True)
    rowarg_p = nc.declare_dram_parameter("rowarg", [P, tiles], dt, isOutput=True)
    accval_p = nc.declare_dram_parameter("accval", [P, G], dt, isOutput=True)
    acct_p = nc.declare_dram_parameter("acct", [P, G], dt, isOutput=True)

    with ExitStack() as ctx:
        def sb(name, shape):
            return ctx.enter_context(nc.sbuf_tensor(name, shape, dt))
        anch_sb = sb("anch_sb", [P, tiles * 4])
        gtb_sb = sb("gtb_sb", [P, (5 + NUM_BASE) * G])
        accval = sb("accval_sb", [P, G])
        acct = sb("acct_sb", [P, G])
        sgn03_sb = sb("sgn03_sb", [P, tiles])
        sgn07_sb = sb("sgn07_sb", [P, tiles])
        rowarg_sb = sb("rowarg_sb", [P, tiles])
        NB = 4  # cross-engine buffers
        # anchor x-coords depend only on the base-anchor type (not the y-block)
        # under x-stripe sharding -> the x min/max tables are per-type consts
        tminxK = [sb(f"tminxK{k}", [P, G]) for k in range(NUM_BASE)]
        tmaxxK = [sb(f"tmaxxK{k}", [P, G]) for k in range(NUM_BASE)]
        tminy = [sb(f"tminy{i}", [P, G]) for i in range(NB)]
        tmaxy = [sb(f"tmaxy{i}", [P, G]) for i in range(NB)]
        iwr = [sb(f"iwr{i}", [P, G]) for i in range(NB)]
        ihr = [sb(f"ihr{i}", [P, G]) for i in range(NB)]
        iw = [sb(f"iw{i}", [P, G]) for i in range(NB)]
        ih = [sb(f"ih{i}", [P, G]) for i in range(NB)]
        inter = [sb(f"inter{i}", [P, G]) for i in range(NB)]
        u = [sb(f"u{i}", [P, G]) for i in range(NB)]
        masks = [sb(f"masks{i}", [P, G]) for i in range(NB)]
        nrq = [sb(f"nrq{i}", [P, 1]) for i in range(NB)]
        cmp_ = sb("cmp", [P, G])
        scr = sb("scr", [P, G])
        sgscr = sb("sgscr", [P, G])
        b03 = sb("b03", [P, 1])
        b07 = sb("b07", [P, 1])

        dma_sem = ctx.enter_context(nc.semaphore("dma_sem"))
        s_dveA = ctx.enter_context(nc.semaphore("s_dveA"))  # DVE front(t) done
        s_gpw = ctx.enter_context(nc.semaphore("s_gpw"))    # GP iwr/ihr(t) done
        s_act1 = ctx.enter_context(nc.semaphore("s_act1"))  # ACT relus(t) done
        s_gpi = ctx.enter_context(nc.semaphore("s_gpi"))    # GP inter/u(t) done
        s_dveU = ctx.enter_context(nc.semaphore("s_dveU"))  # DVE back(t) done
        s_act2 = ctx.enter_context(nc.semaphore("s_act2"))  # ACT signs(t) done
        s_dveR = ctx.enter_context(nc.semaphore("s_dveR"))  # DVE nrq(t) ready
        s_act3 = ctx.enter_context(nc.semaphore("s_act3"))  # ACT masksign(t) done
        s_dveE = ctx.enter_context(nc.semaphore("s_dveE"))  # DVE enc(t) done
        block = ctx.enter_context(nc.Block())

        gx1 = gtb_sb[:, 0 * G:1 * G]
        gy1 = gtb_sb[:, 1 * G:2 * G]
        gx2 = gtb_sb[:, 2 * G:3 * G]
        gy2 = gtb_sb[:, 3 * G:4 * G]

        @block.sync
        def _(sync):
            sync.dma_start(out=anch_sb[:], in_=anch_p[:]).then_inc(dma_sem, 16)
            sync.dma_start(out=gtb_sb[:], in_=gtb_p[:]).then_inc(dma_sem, 16)
            sync.wait_ge(s_dveU, tiles)
            sync.wait_ge(s_act2, tiles)
            sync.wait_ge(s_dveE, tiles)
            sync.dma_start(out=rowarg_p[:], in_=rowarg_sb[:]).then_inc(dma_sem, 16)
            sync.dma_start(out=sgn03_p[:], in_=sgn03_sb[:]).then_inc(dma_sem, 16)
            sync.dma_start(out=sgn07_p[:], in_=sgn07_sb[:]).then_inc(dma_sem, 16)
            sync.dma_start(out=accval_p[:], in_=accval[:]).then_inc(dma_sem, 16)
            sync.dma_start(out=acct_p[:], in_=acct[:]).then_inc(dma_sem, 16)

        @block.gpsimd
        def _(gp):
            gp.wait_ge(dma_sem, 32)
            for t in range(tiles + 1):
                if t < tiles:
                    b = t % NB
                    lo, hi = windows[t]
                    w = hi - lo
                    k = t // TPK
                    gp.wait_ge(s_dveA, t + 1)
                    if t >= NB:
                        gp.wait_ge(s_act1, t - NB + 1)  # ACT read iwr of t-NB
                    gp.tensor_tensor(out=iwr[b][:, :w],
                                     in0=tminxK[k][:, lo:hi],
                                     in1=tmaxxK[k][:, lo:hi], op=op.subtract)
                    gp.tensor_tensor(out=ihr[b][:, :w], in0=tminy[b][:, :w],
                                     in1=tmaxy[b][:, :w],
                                     op=op.subtract).then_inc(s_gpw, 1)
                if t >= 1:
                    tq = t - 1
                    b = tq % NB
                    lo, hi = windows[tq]
                    w = hi - lo
                    k = tq // TPK
                    rsw = gtb_sb[:, (5 + k) * G + lo:(5 + k) * G + hi]
                    gp.wait_ge(s_act1, tq + 1)
                    if tq >= NB:
                        gp.wait_ge(s_dveU, tq - NB + 1)  # DVE read u of tq-NB
                        gp.wait_ge(s_act2, tq - NB + 1)  # ACT signed u of tq-NB
                    gp.tensor_tensor(out=inter[b][:, :w], in0=iw[b][:, :w],
                                     in1=ih[b][:, :w], op=op.mult)
                    gp.tensor_tensor(out=u[b][:, :w], in0=inter[b][:, :w],
                                     in1=rsw, op=op.mult).then_inc(s_gpi, 1)

        @block.vector
        def _(vector):
            vector.wait_ge(dma_sem, 32)
            vector.memset(accval[:], -1.0)
            vector.memset(acct[:], 0.0)
            vector.memset(b03[:], -C03)
            vector.memset(b07[:], -C07)
            gx1 = gtb_sb[:, 0 * G:1 * G]
            gx2 = gtb_sb[:, 2 * G:3 * G]
            for k in range(NUM_BASE):
                t0 = k * TPK
                ax1 = anch_sb[:, t0 * 4 + 0:t0 * 4 + 1]
                ax2 = anch_sb[:, t0 * 4 + 2:t0 * 4 + 3]
                vector.tensor_scalar(out=tminxK[k][:], in0=gx2, scalar1=ax2,
                                     scalar2=None, op0=op.min)
                vector.tensor_scalar(out=tmaxxK[k][:], in0=gx1, scalar1=ax1,
                                     scalar2=None, op0=op.max)
            for t in range(tiles + 3):
                if t < tiles:
                    b = t % NB
                    lo, hi = windows[t]
                    w = hi - lo
                    ay1 = anch_sb[:, t * 4 + 1:t * 4 + 2]
                    ay2 = anch_sb[:, t * 4 + 3:t * 4 + 4]
                    gy1w = gtb_sb[:, 1 * G + lo:1 * G + hi]
                    gy2w = gtb_sb[:, 3 * G + lo:3 * G + hi]
                    if t >= NB and t % NB == 0:
                        # one guard covers the next NB fronts: front(t+NB-1)
                        # needs GP first(t-1) done <=> s_gpw >= t
                        vector.wait_ge(s_gpw, t)
                    vector.tensor_scalar(out=tminy[b][:, :w], in0=gy2w,
                                         scalar1=ay2, scalar2=None, op0=op.min)
                    vector.tensor_scalar(out=tmaxy[b][:, :w], in0=gy1w,
                                         scalar1=ay1, scalar2=None,
                                         op0=op.max).then_inc(s_dveA, 1)
                if 2 <= t < tiles + 2:
                    tp = t - 2
                    b = tp % NB
                    lo, hi = windows[tp]
                    w = hi - lo
                    vector.wait_ge(s_gpi, tp + 1)
                    # negated rowmax (the ACT mask bias) straight from the
                    # reduce via negate=True — no separate negation op
                    vector.tensor_reduce(out=nrq[b][:], in_=u[b][:, :w],
                                         axis=mybir.AxisListType.X, op=op.max,
                                         negate=True).then_inc(s_dveR, 1)
                    vector.tensor_tensor(out=cmp_[:, :w], in0=u[b][:, :w],
                                         in1=accval[:, lo:hi], op=op.is_gt)
                    vector.scalar_tensor_tensor(out=acct[:, lo:hi],
                                                in0=cmp_[:, :w],
                                                scalar=float(tp),
                                                in1=acct[:, lo:hi],
                                                op0=op.mult, op1=op.max)
                    vector.tensor_tensor(out=accval[:, lo:hi],
                                           in0=u[b][:, :w],
                                           in1=accval[:, lo:hi],
                                           op=op.max).then_inc(s_dveU, 1)
                if t >= 3:
                    te = t - 3
                    be = te % NB
                    lo, hi = windows[te]
                    w = hi - lo
                    bmiw = gtb_sb[:, 4 * G + lo:4 * G + hi]
                    if te % 2 == 0:
                        # one wait covers this enc and the next
                        vector.wait_ge(s_act3, min(te + 2, tiles))
                    # enc = sum(sign(u-rowmax) * bmi); host adds the window's
                    # bmi sum (sign is {-1,0}, zero exactly at the rowmax)
                    vector.scalar_tensor_tensor(
                        out=scr[:, :w], in0=masks[be][:, :w], scalar=1.0,
                        in1=bmiw, op0=op.mult, op1=op.mult,
                        accum_out=rowarg_sb[:, te:te + 1]).then_inc(s_dveE, 1)

        @block.scalar
        def _(sc):
            for t in range(tiles + 1):
                if t < tiles:
                    b = t % NB
                    w = windows[t][1] - windows[t][0]
                    sc.wait_ge(s_gpw, t + 1)
                    sc.activation(out=iw[b][:, :w], in_=iwr[b][:, :w],
                                  func=act.Relu, bias=1.0, scale=1.0)
                    sc.activation(out=ih[b][:, :w], in_=ihr[b][:, :w],
                                  func=act.Relu, bias=1.0,
                                  scale=1.0).then_inc(s_act1, 1)
                if t >= 1:
                    tr = t - 1
                    b = tr % NB
                    w = windows[tr][1] - windows[tr][0]
                    sc.wait_ge(s_gpi, tr + 1)
                    sc.wait_ge(s_dveR, tr + 1)
                    if tr >= NB:
                        sc.wait_ge(s_dveE, tr - NB + 1)  # enc read masks(tr-NB)
                    sc.activation(out=masks[b][:, :w], in_=u[b][:, :w],
                                  func=act.Sign, bias=nrq[b][:, 0:1],
                                  scale=1.0).then_inc(s_act3, 1)
                    sc.activation(out=sgscr[:, :w], in_=u[b][:, :w],
                                  func=act.Sign, bias=b03[:, 0:1], scale=1.0,
                                  accum_out=sgn03_sb[:, tr:tr + 1])
                    sc.activation(out=sgscr[:, :w], in_=u[b][:, :w],
                                  func=act.Sign, bias=b07[:, 0:1],
                                  scale=1.0,
                                  accum_out=sgn07_sb[:, tr:tr + 1]).then_inc(
                                      s_act2, 1)
            # final inc only via s_act2 (sync waits on the counting sems)

    return nc


# --------------------------------------------------------------- epilogue --
def _subsample(labels, target_value, max_count, U):
    is_t = labels == np.float32(target_value)
    pri = np.where(is_t, U, np.float32(-1.0)).astype(np.float32)
    order = np.argsort(-pri, kind="stable")
    rank = np.empty(labels.shape[0], np.int64)
    rank[order] = np.arange(labels.shape[0])
    drop = is_t & (rank >= max_count)
    return np.where(drop, np.float32(-1.0), labels).astype(np.float32)


def _bbox_transform(ex, gt):
    one = np.float32(1.0)
    half = np.float32(0.5)
    ew = ex[:, 2] - ex[:, 0] + one
    eh = ex[:, 3] - ex[:, 1] + one
    ecx = ex[:, 0] + half * ew
    ecy = ex[:, 1] + half * eh
    gw = gt[:, 2] - gt[:, 0] + one
    gh = gt[:, 3] - gt[:, 1] + one
    gcx = gt[:, 0] + half * gw
    gcy = gt[:, 1] + half * gh
    dx = (gcx - ecx) / ew
    dy = (gcy - ecy) / eh
    dw = np.log(gw / ew)
    dh = np.log(gh / eh)
    return np.stack([dx, dy, dw, dh], axis=1).astype(np.float32)


# ----------------------------------------------------------------- kernel --
def kernel(scores, gt_boxes, metadata, _trace=False):
    global KERNEL_EXEC_NS, KERNEL_PROFILE
    from concourse.bass_utils import run_bass_kernel_spmd

    trace = _trace or os.environ.get("ANCHOR_KERNEL_TRACE") == "1"
    if trace:
        try:
            import antenv.axon_hooks  # noqa: F401  (shimmed by test.py)
        except ImportError:
            trace = False

    gt = np.asarray(gt_boxes, F32)[0]                       # (256, 4)
    meta = np.asarray(metadata, F32)[0]

    anchors = _all_anchors()                                # (N, 4) f32
    base = _base_anchors()
    one = np.float32(1.0)
    area_a = ((anchors[:, 2] - anchors[:, 0] + one) *
              (anchors[:, 3] - anchors[:, 1] + one)).astype(F32)
    area_b = ((gt[:, 2] - gt[:, 0] + one) *
              (gt[:, 3] - gt[:, 1] + one)).astype(F32)
    A9 = area_a[:NUM_BASE]                                  # one area per type

    pi, wins = _gt_windows(gt, base)
    wt = np.array([hi - lo for lo, hi in wins], np.float32)  # per-tile widths

    # 1/S rows (gt axis permuted by pi), correctly rounded f32 of 1/fl(A_k+B_g)
    S9 = (A9[:, None] + area_b[None, pi]).astype(F32)       # (9, G)
    RS9 = (1.0 / S9.astype(np.float64)).astype(F32)
    gperm = gt[pi]
    bmi_row = (np.float32(BIG) - pi.astype(F32))            # BIG - original id

    grow = np.concatenate([gperm[:, 0], gperm[:, 1], gperm[:, 2], gperm[:, 3],
                           bmi_row, RS9.ravel()])
    gtb_arr = np.ascontiguousarray(
        np.broadcast_to(grow, (P, (5 + NUM_BASE) * G))).astype(F32)

    in_maps = []
    for c in range(N_CORES):
        asl = anchors[SEL[c]]                               # (TILES, P, 4)
        anch_arr = np.ascontiguousarray(
            asl.transpose(1, 0, 2).reshape(P, TILES * 4)).astype(F32)
        in_maps.append({"anch": anch_arr, "gtb": gtb_arr})

    nc = _build_bass(windows=wins)
    res = run_bass_kernel_spmd(nc, in_maps, core_ids=list(range(N_CORES)),
                               trace=trace)
    if trace:
        KERNEL_EXEC_NS = res.exec_time_ns
        KERNEL_PROFILE = res.profile_json
    outs = res.results

    def flat(name):
        # [128, TILES] per core, device position (c,t,p) -> global anchor
        X = np.empty(N, F32)
        for c in range(N_CORES):
            X[SEL[c].reshape(-1)] = np.asarray(outs[c][name]).T.ravel()
        return X

    sgn03 = flat("sgn03")
    sgn07 = flat("sgn07")
    # device encodes sum(sign(u - rowmax) * bmi) with sign in {-1, 0};
    # adding the window's bmi sum recovers the single-hot BIG - gt_id code
    sb_tile = np.array([bmi_row[lo:hi].astype(np.float64).sum()
                        for lo, hi in wins])
    sb_anchor = np.empty(N, np.float64)
    for c in range(N_CORES):
        sb_anchor[SEL[c].reshape(-1)] = np.repeat(sb_tile, P)
    rowarg_enc = flat("rowarg").astype(np.float64) + sb_anchor
    # per-anchor window width (by its tile)
    wt_anchor = np.empty(N, F32)
    for c in range(N_CORES):
        wt_anchor[SEL[c].reshape(-1)] = np.repeat(wt, P)

    argmax_inds = np.rint(np.float32(BIG) - rowarg_enc).astype(np.int64)
    # enc outside the single-hot range means a tied (e.g. all-zero) row ->
    # first-occurrence argmax is 0
    argmax_inds[(argmax_inds < 0) | (argmax_inds > G - 1)] = 0

    # global per-gt argmax: max value, tie -> smallest original anchor index
    V = np.stack([np.asarray(outs[c]["accval"]) for c in range(N_CORES)])
    T = np.rint(np.stack([np.asarray(outs[c]["acct"])
                          for c in range(N_CORES)])).astype(np.int64)
    AIDX = np.empty((N_CORES, P, G), np.int64)
    for c in range(N_CORES):
        AIDX[c] = SEL[c][T[c], np.arange(P)[:, None]]
    best = V.max(axis=(0, 1))
    cand = np.where(V == best[None, None, :], AIDX, N)
    gt_argmax_dev = cand.min(axis=(0, 1))                   # indexed by pi-order
    gt_argmax = np.empty(G, np.int64)
    gt_argmax[pi] = gt_argmax_dev                           # original gt order

    # ---- label assembly (order follows the reference exactly) ----
    labels = np.full(N, -1.0, F32)
    labels = np.where(sgn03 <= -wt_anchor + 0.5, np.float32(0.0), labels)
    labels[gt_argmax] = 1.0
    labels = np.where(sgn07 > -wt_anchor + 0.5, np.float32(1.0), labels)

    import jax
    cpu = jax.devices("cpu")[0]
    with jax.default_device(cpu):
        k = jax.random.key(42)
        kpos, kneg = jax.random.split(k)
        U1 = np.asarray(jax.random.uniform(kpos, (N,)), F32)
        U2 = np.asarray(jax.random.uniform(kneg, (N,)), F32)

    num_fg = int(RPN_FG_FRACTION * RPN_BATCHSIZE)
    labels = _subsample(labels, 1.0, num_fg, U1)
    num_bg = RPN_BATCHSIZE - int(np.sum(labels == np.float32(1.0)))
    labels = _subsample(labels, 0.0, num_bg, U2)

    targets = _bbox_transform(anchors, gt[argmax_inds])

    inside = ((anchors[:, 0] >= 0) & (anchors[:, 1] >= 0) &
              (anchors[:, 2] < meta[1]) & (anchors[:, 3] < meta[0]))
    labels = np.where(inside, labels, np.float32(-1.0)).astype(F32)

    return labels[None, :], targets[None, :, :]
